# revision 45
# baseline (speedup 1.0000x reference)
"""Trainium2 Bass kernel: 8-layer ternary (BitNet-1.58) dense transformer.

Model (per reference):
    h = embed[input_ids]                                  # (B=2, S=1024, H=2048)
    8x: y = h @ ternary(W_l)^T + b_l ; h = LN(y + h)*g+b  # H=2048
    h = LN(h)*final_g + final_b
    logits = h @ ternary(head_W)^T                        # (B, S, V=32000)

Sharding over 8 NeuronCores: fully data-parallel over the 2048 tokens
(256 tokens/core). Each core streams the full layer weights (fp8, 33 MB)
during the layer phase and the full lm_head (fp8, 65 MB) during the head
phase; the head stays compute-bound (2.3x margin over DMA at 358 GB/s).
There are NO collectives — cores never exchange data, which removes the
AllGather straggler coupling (per-core clock-throttle variance made a
tensor-parallel head's gather completion highly variable) and the
collective bootstrap barrier (whose ring ownership blocks XBAR
transposes).

Precision: ternary weights are scaled by an exact power of two (2^-e, e~6)
so they are EXACT in fp8e4; the compensating factor (s * 2^e ~= 1) is folded
into the activation cast. Matmuls run fp16 (activations, stationary) x
fp8 (weights, moving); PSUM accumulates fp32; the residual/LN path stays
fp32. Logits are written bf16 and upcast on the host.

Scheduling:
  - The PE runs matmuls ONLY. Activation transposes go through the DMA XBAR
    (dma_start(transpose=True), fp16, SBUF->SBUF, out[p, kt, tok] =
    in[tok, kt*128 + p] — verified against CoreSim's InstDmaTransposeAnt),
    alternating between the two HWDGE queues (sync/scalar) by tile parity.
  - Software pipelining by emission order: each tile's next-layer cast is
    emitted directly after its ln_finish, so the (strict-FIFO) vector engine
    runs it before the other tile's LN chain, and the XBAR transpose lands
    during the other tile's matmuls. Layer-0 transposed inputs come from
    the host ("h0T"), so the first matmul waits only on two small DMAs.
  - Weights ride the scalar-engine DGE queue; transposes and small loads
    ride the sync queue.
  - The board power governor caps sustained near-100%-duty matmul streams
    at K=13/16 (~1.95 GHz, type-31 HAM events) with large run-to-run
    variance; minimizing total PE cycles is what matters, so transposes are
    kept off the PE even though the PE has idle slots for them.
"""

import os
import sys

import numpy as np

try:
    import concourse.bass as bass
except ImportError:  # grading container should have it on sys.path already
    sys.path.insert(0, "/opt/trn_rl_repo")
    import concourse.bass as bass

import ml_dtypes
import concourse.mybir as mybir
import concourse.tile as tile
from concourse import bacc
from concourse.bass_utils import run_bass_kernel_spmd
from contextlib import ExitStack

F32 = mybir.dt.float32
BF16 = mybir.dt.bfloat16
FP16 = mybir.dt.float16
FP8 = mybir.dt.float8e4
AX = mybir.AxisListType
OP = mybir.AluOpType
AF = mybir.ActivationFunctionType
EPS = 1e-5

# Full-size problem config (B=2, S=1024 -> 2048 tokens).
# CV: vocab chunk streamed per head step.
CFG_FULL = dict(L=8, H=2048, NTOK=2048, NC=8, TT=2, V=32000, NV=500, CH=512,
                CV=2000)


def build_nc(cfg, sigmas, head_sigma, use_gb):
    L, H, NTOK, NC, TT = cfg["L"], cfg["H"], cfg["NTOK"], cfg["NC"], cfg["TT"]
    V, NV, CH, CV = cfg["V"], cfg["NV"], cfg["CH"], cfg["CV"]
    KT = H // 128
    KH = KT // 2  # k-tiles per weight half
    NCH = H // CH
    NVC = CV // NV  # vocab sub-chunks per streamed chunk (4)
    NCHK = V // CV  # streamed head chunks (16)
    TPC = TT * 128
    assert NTOK == NC * TPC

    nc = bacc.Bacc("TRN2", target_bir_lowering=False, debug=False, num_devices=NC)
    h0 = nc.declare_dram_parameter("h0", [TT, 128, H], BF16, isOutput=False)
    h0T = nc.declare_dram_parameter("h0T", [TT, 128, KT, 128], FP16, isOutput=False)
    w_ = nc.declare_dram_parameter("w", [L, KT, 128, H], FP8, isOutput=False)
    if use_gb:
        lng = nc.declare_dram_parameter("lng", [L, H], BF16, isOutput=False)
        lnb = nc.declare_dram_parameter("lnb", [L, H], BF16, isOutput=False)
        lbias = nc.declare_dram_parameter("lbias", [L, H], BF16, isOutput=False)
        fing = nc.declare_dram_parameter("fing", [H], BF16, isOutput=False)
        finb = nc.declare_dram_parameter("finb", [H], BF16, isOutput=False)
    hw_ = nc.declare_dram_parameter("hw", [KT, 128, V], FP8, isOutput=False)
    eps_d = nc.declare_dram_parameter("eps", [128, 1], F32, isOutput=False)
    out = nc.declare_dram_parameter("out", [TPC, V], BF16, isOutput=True)

    with tile.TileContext(nc) as tc:
        with ExitStack() as ctxA:
            consts = ctxA.enter_context(tc.tile_pool(name="consts", bufs=1))
            state = ctxA.enter_context(tc.tile_pool(name="state", bufs=3))
            zpool = ctxA.enter_context(tc.tile_pool(name="z", bufs=2))
            hscp = ctxA.enter_context(tc.tile_pool(name="hsc", bufs=2))
            hTp = ctxA.enter_context(tc.tile_pool(name="hT", bufs=2))
            wp = ctxA.enter_context(tc.tile_pool(name="w", bufs=8))
            hwp = ctxA.enter_context(tc.tile_pool(name="hw", bufs=2))
            outp = ctxA.enter_context(tc.tile_pool(name="outstg", bufs=4))
            gbp = None
            if use_gb:
                gbp = ctxA.enter_context(tc.tile_pool(name="gb", bufs=2))
            smp = ctxA.enter_context(tc.tile_pool(name="small", bufs=16))
            psY = ctxA.enter_context(
                tc.tile_pool(name="psY", bufs=2 * NCH, space="PSUM")
            )

            eps_t = consts.tile([128, 1], F32)
            nc.sync.dma_start(eps_t[:], eps_d[:])

            def load_w(l):
                """Layer weights in 4-ktile quarters — fine granularity lets
                each layer's first matmuls start before the whole 4.2 MB
                layer arrives (the early phase is DMA-bandwidth-starved).
                All on the scalar queue: mixing weight DMAs onto the sync
                queue alongside XBAR transposes corrupts transfers.
                """
                qs = []
                for qf in range(4):
                    wt = wp.tile([128, KT // 4, H], FP8, tag="w", name=f"w{l}_{qf}")
                    # alternate quarters across both HWDGE queues; the sync
                    # queue carries only plain DMAs (XBARs live on scalar —
                    # mixing XBAR + bulk DMAs on sync corrupts transfers)
                    eng = nc.scalar if qf % 2 == 0 else nc.sync
                    eng.dma_start(
                        wt[:],
                        w_[l, qf * (KT // 4) : (qf + 1) * (KT // 4)].rearrange(
                            "k p o -> p k o"
                        ),
                    )
                    qs.append(wt)
                return qs

            w_half0 = load_w(0)

            h_cur = []
            hT = []
            for t in range(TT):
                # layer-0 transposed input comes precomputed from the host
                ht = hTp.tile([128, KT, 128], FP16, tag="hT", name=f"hTp{t}")
                nc.sync.dma_start(ht[:], h0T[t])
                hT.append(ht)
                st = state.tile([128, H], BF16, name=f"hinit{t}", tag="state")
                h_cur.append(st)
            # residual loads: plenty of slack (first needed ~35us in)
            for t in range(TT):
                nc.scalar.dma_start(h_cur[t][:], h0[t])
            w_half1 = load_w(1)

            def cast_transpose(src_f32, scale_imm, pool, name, t):
                """h [128tok, H] f32 -> hT [128, KT, 128tok] fp16 * scale."""
                hsc = hscp.tile([128, H], FP16, tag="hsc", name=f"hsc{name}")
                nc.vector.tensor_scalar_mul(hsc[:], src_f32[:], float(scale_imm))
                dst = pool.tile([128, KT, 128], FP16, tag="hT", name=f"hT{name}")
                nc.scalar.dma_start(dst[:], hsc[:], transpose=True)
                return dst

            def ln_finish(affine_src, S_ap, SS_ap, g_t, b_t, name):
                S = smp.tile([128, 1], F32, tag="s0", name=f"S{name}")
                SS = smp.tile([128, 1], F32, tag="s1", name=f"SS{name}")
                nc.vector.tensor_reduce(S[:], S_ap, axis=AX.X, op=OP.add)
                nc.vector.tensor_reduce(SS[:], SS_ap, axis=AX.X, op=OP.add)
                negmean = smp.tile([128, 1], F32, tag="s2", name=f"nm{name}")
                nc.vector.tensor_scalar_mul(negmean[:], S[:], -1.0 / H)
                msq = smp.tile([128, 1], F32, tag="s3", name=f"msq{name}")
                nc.vector.tensor_scalar_mul(msq[:], SS[:], 1.0 / H)
                var = smp.tile([128, 1], F32, tag="s4", name=f"var{name}")
                nc.vector.tensor_tensor(var[:], negmean[:], negmean[:], OP.mult)
                nc.vector.tensor_tensor(var[:], msq[:], var[:], OP.subtract)
                std = smp.tile([128, 1], F32, tag="s5", name=f"std{name}")
                nc.scalar.activation(std[:], var[:], AF.Sqrt, bias=eps_t[:])
                rstd = smp.tile([128, 1], F32, tag="s6", name=f"rstd{name}")
                nc.vector.reciprocal(rstd[:], std[:])
                hn = state.tile([128, H], F32, tag="state", name=f"h{name}")
                nc.vector.tensor_scalar(
                    hn[:], affine_src[:], negmean[:], rstd[:], OP.add, OP.mult
                )
                if g_t is not None:
                    nc.vector.tensor_tensor(hn[:], hn[:], g_t[:], OP.mult)
                    nc.vector.tensor_tensor(hn[:], hn[:], b_t[:], OP.add)
                return hn

            fg = fb = None
            if use_gb:
                fg = gbp.tile([128, H], BF16, tag="g", name="gfin")
                nc.scalar.dma_start(fg[:], fing[None, :].to_broadcast((128, H)))
                fb = gbp.tile([128, H], BF16, tag="b", name="bfin")
                nc.scalar.dma_start(fb[:], finb[None, :].to_broadcast((128, H)))

            hTfin = [None] * TT

            def emit_final(t):
                """Final LN -> transposed head input for tile t.

                When the final affine is trivial (this input), LN of an
                LN output is the identity to ~1e-6 relative — skip it and
                transpose the layer-8 LN output directly.
                """
                h8 = h_cur[t]
                if not use_gb:
                    hTfin[t] = cast_transpose(h8, head_sigma, hTp, f"fin{t}", t)
                    return
                if t == 0:
                    fsums = smp.tile(
                        [128, 1 + NCH], F32, tag="fsums", name=f"smfin{t}"
                    )
                    nc.vector.tensor_reduce(
                        fsums[:, 0:1], h8[:], axis=AX.X, op=OP.add
                    )
                    for i in range(NCH):
                        dump = psY.tile(
                            [128, CH], F32, tag="psY", name=f"dmp{t}_{i}"
                        )
                        nc.scalar.activation(
                            dump[:],
                            h8[:, i * CH : (i + 1) * CH],
                            AF.Square,
                            accum_out=fsums[:, 1 + i : 2 + i],
                        )
                    hfin = ln_finish(
                        h8, fsums[:, 0:1], fsums[:, 1 : 1 + NCH],
                        fg, fb, f"fin{t}",
                    )
                else:
                    fsums = smp.tile([128, 2], F32, tag="fsums1", name=f"smfin{t}")
                    nc.vector.tensor_reduce(
                        fsums[:, 0:1], h8[:], axis=AX.X, op=OP.add
                    )
                    fsq = zpool.tile([128, H], F32, tag="z", name=f"fsq{t}")
                    nc.vector.tensor_tensor(fsq[:], h8[:], h8[:], OP.mult)
                    nc.vector.tensor_reduce(
                        fsums[:, 1:2], fsq[:], axis=AX.X, op=OP.add
                    )
                    hfin = ln_finish(
                        h8, fsums[:, 0:1], fsums[:, 1:2], fg, fb, f"fin{t}"
                    )
                hTfin[t] = cast_transpose(hfin, head_sigma, hTp, f"fin{t}", t)

            gbt = {}
            hw_pre = []  # first head chunks, prefetched under the last layer
            for l in range(L):
                if l > 1:
                    w_half = load_w(l)
                elif l == 1:
                    w_half = w_half1
                else:
                    w_half = w_half0
                if l == L - 1:
                    for ch in range(2):
                        hwt = hwp.tile(
                            [128, KT, CV], FP8, tag="hw", name=f"hw{ch}"
                        )
                        nc.scalar.dma_start(
                            hwt[:],
                            hw_[:, :, ch * CV : (ch + 1) * CV].rearrange(
                                "k p v -> p k v"
                            ),
                        )
                        hw_pre.append(hwt)
                if use_gb:
                    g_t = gbp.tile([128, H], BF16, tag="g", name=f"g{l}")
                    nc.scalar.dma_start(g_t[:], lng[l][None, :].to_broadcast((128, H)))
                    b_t = gbp.tile([128, H], BF16, tag="b", name=f"b{l}")
                    nc.scalar.dma_start(b_t[:], lnb[l][None, :].to_broadcast((128, H)))
                    bias_t = gbp.tile([128, H], BF16, tag="bias", name=f"bias{l}")
                    nc.scalar.dma_start(
                        bias_t[:], lbias[l][None, :].to_broadcast((128, H))
                    )
                    gbt = dict(g=g_t, b=b_t, bias=bias_t)

                for t in range(TT):
                    hTt = hT[t]
                    ps = []
                    for i in range(NCH):
                        p = psY.tile([128, CH], F32, tag="psY", name=f"ps{l}_{t}_{i}")
                        ps.append(p)
                    for kt in range(KT):
                        wt = w_half[kt // (KT // 4)]
                        for i in range(NCH):
                            nc.tensor.matmul(
                                ps[i][:],
                                lhsT=hTt[:, kt, :],
                                rhs=wt[:, kt % (KT // 4), i * CH : (i + 1) * CH],
                                start=(kt == 0),
                                stop=(kt == KT - 1),
                            )
                    z = zpool.tile([128, H], F32, tag="z", name=f"z{l}_{t}")
                    sums = smp.tile([128, 1 + NCH], F32, tag="sums", name=f"sm{l}_{t}")
                    resid = h_cur[t]
                    if use_gb:
                        hb = zpool.tile([128, H], F32, tag="hb", name=f"hb{l}_{t}")
                        nc.vector.tensor_tensor(hb[:], h_cur[t][:], gbt["bias"][:], OP.add)
                        resid = hb
                    for i in range(NCH):
                        nc.vector.tensor_add(
                            z[:, i * CH : (i + 1) * CH],
                            ps[i][:],
                            resid[:, i * CH : (i + 1) * CH],
                        )
                    nc.vector.tensor_reduce(sums[:, 0:1], z[:], axis=AX.X, op=OP.add)
                    for i in range(NCH):
                        nc.scalar.activation(
                            ps[i][:],
                            z[:, i * CH : (i + 1) * CH],
                            AF.Square,
                            accum_out=sums[:, 1 + i : 2 + i],
                        )
                    h_cur[t] = ln_finish(
                        z, sums[:, 0:1], sums[:, 1 : 1 + NCH],
                        gbt.get("g"), gbt.get("b"), f"{l}_{t}",
                    )
                    if l + 1 < L:
                        # emitted NOW: the cast sits right behind this tile's
                        # LN in the vector FIFO and the XBAR transpose runs
                        # during the other tile's matmuls
                        hT[t] = cast_transpose(
                            h_cur[t], sigmas[l + 1], hTp, f"{l + 1}_{t}", t
                        )
                    else:
                        emit_final(t)

            # head: stream the full lm_head in CV-wide vocab chunks; each
            # chunk serves both token tiles (compute:DMA ~ 2.3:1)
            for ch in range(NCHK):
                if ch < 2:
                    hwt = hw_pre[ch]
                else:
                    hwt = hwp.tile([128, KT, CV], FP8, tag="hw", name=f"hw{ch}")
                    nc.scalar.dma_start(
                        hwt[:],
                        hw_[:, :, ch * CV : (ch + 1) * CV].rearrange("k p v -> p k v"),
                    )
                for t in range(TT):
                    pss = [
                        psY.tile([128, CH], F32, tag="psY", name=f"ph{ch}_{t}_{v}")
                        for v in range(NVC)
                    ]
                    for kt in range(KT):
                        for vi in range(NVC):
                            nc.tensor.matmul(
                                pss[vi][:, 0:NV],
                                lhsT=hTfin[t][:, kt, :],
                                rhs=hwt[:, kt, vi * NV : (vi + 1) * NV],
                                start=(kt == 0),
                                stop=(kt == KT - 1),
                                skip_group_check=True,
                            )
                    o_t = outp.tile([128, CV], BF16, tag="ostg", name=f"o{ch}_{t}")
                    for vi in range(NVC):
                        nc.scalar.copy(
                            o_t[:, vi * NV : (vi + 1) * NV], pss[vi][:, 0:NV]
                        )
                    nc.sync.dma_start(
                        out[t * 128 : (t + 1) * 128, ch * CV : (ch + 1) * CV],
                        o_t[:],
                    )

    return nc


def _ternary(wmat):
    """Exact {-1,0,1} ternary tensor + fp32 scale, matching the reference."""
    w = np.asarray(wmat, dtype=np.float32)
    s = np.mean(np.abs(w), dtype=np.float32)
    t = np.clip(np.rint(w / (s + np.float32(1e-8))), -1.0, 1.0).astype(np.float32)
    return t, float(s)


def _split_scale(s):
    """s = sigma * 2^-e with sigma ~ 1 and 2^-e exact in fp8e4."""
    e = int(np.clip(np.round(-np.log2(s)), -7, 9))
    return s * (2.0**e), e


_NC_CACHE = {}
_LAST_RESULTS = None


def kernel(**inputs):
    global _LAST_RESULTS
    cfg = CFG_FULL
    L, H, NTOK, NC, TT, V = (
        cfg["L"], cfg["H"], cfg["NTOK"], cfg["NC"], cfg["TT"], cfg["V"],
    )
    KT = H // 128
    TPC = TT * 128  # tokens per core
    BF = ml_dtypes.bfloat16
    F8 = ml_dtypes.float8_e4m3fn

    ids = np.asarray(inputs["input_ids"]).astype(np.int64).reshape(-1)
    embed = np.asarray(inputs["embed"], dtype=np.float32)
    layer_w = np.asarray(inputs["layer_w"], dtype=np.float32)
    layer_b = np.asarray(inputs["layer_b"], dtype=np.float32)
    ln_g = np.asarray(inputs["ln_g"], dtype=np.float32)
    ln_b = np.asarray(inputs["ln_b"], dtype=np.float32)
    final_g = np.asarray(inputs["final_g"], dtype=np.float32)
    final_b = np.asarray(inputs["final_b"], dtype=np.float32)
    head_w = np.asarray(inputs["head_w"], dtype=np.float32)

    use_gb = not (
        np.all(layer_b == 0.0)
        and np.all(ln_g == 1.0)
        and np.all(ln_b == 0.0)
        and np.all(final_g == 1.0)
        and np.all(final_b == 0.0)
    )

    h0_full = embed[ids]  # [NTOK, H] fp32

    sigmas = []
    wT = np.empty([L, KT, 128, H], dtype=F8)
    for l in range(L):
        t, s = _ternary(layer_w[l])
        sig, e = _split_scale(s)
        sigmas.append(sig)
        wT[l] = (
            (np.ascontiguousarray(t.T) * np.float32(2.0**-e))
            .reshape(KT, 128, H)
            .astype(F8)
        )
    th, head_scale = _ternary(head_w)
    head_sigma, e_h = _split_scale(head_scale)
    headT = (
        (np.ascontiguousarray(th.T) * np.float32(2.0**-e_h))
        .reshape(KT, 128, -1)
        .astype(F8)
    )  # [KT, 128, V]

    key = (id(cfg), tuple(sigmas), head_sigma, use_gb)
    if key not in _NC_CACHE:
        _NC_CACHE.clear()
        nc = build_nc(cfg, sigmas, head_sigma, use_gb)
        # Bacc.finalize runs the TRN2 legalization passes (1-wait-per-
        # instruction event-semaphore split, matmul->ldweights wait motion,
        # register allocation). The PJRT exec path serializes nc as-is.
        nc.finalize()
        _NC_CACHE[key] = nc
    nc = _NC_CACHE[key]

    common = {
        "w": wT,
        "hw": headT,
        "eps": np.full((128, 1), EPS, np.float32),
    }
    if use_gb:
        common.update(
            lng=ln_g.astype(BF),
            lnb=ln_b.astype(BF),
            lbias=layer_b.astype(BF),
            fing=final_g.astype(BF),
            finb=final_b.astype(BF),
        )
    in_maps = []
    for c in range(NC):
        h0c = np.ascontiguousarray(
            h0_full[c * TPC : (c + 1) * TPC].reshape(TT, 128, H)
        )
        # layer-0 transposed+scaled input, in XBAR block layout
        h0t = np.ascontiguousarray(
            (h0c * np.float32(sigmas[0]))
            .transpose(0, 2, 1)  # [TT, H, 128tok]
            .reshape(TT, KT, 128, 128)
            .transpose(0, 2, 1, 3)  # [TT, 128p, KT, 128tok]
        ).astype(np.float16)
        in_maps.append(dict(common, h0=h0c.astype(BF), h0T=h0t))

    trace = bool(int(os.environ.get("TRIKERNEL_TRACE", "0")))
    res = run_bass_kernel_spmd(nc, in_maps, core_ids=list(range(NC)), trace=trace)
    _LAST_RESULTS = res

    full = np.concatenate(
        [np.asarray(res.results[c]["out"]) for c in range(NC)], axis=0
    )  # [NTOK, V] bf16
    return full.reshape(2, 1024, 32000).astype(np.float32)


# revision 53
# speedup vs baseline: 1.0387x; 1.0387x over previous
"""Trainium2 Bass kernel: 8-layer ternary (BitNet-1.58) dense transformer.

Model (per reference):
    h = embed[input_ids]                                  # (B=2, S=1024, H=2048)
    8x: y = h @ ternary(W_l)^T + b_l ; h = LN(y + h)*g+b  # H=2048
    h = LN(h)*final_g + final_b
    logits = h @ ternary(head_W)^T                        # (B, S, V=32000)

Sharding over 8 NeuronCores: fully data-parallel over the 2048 tokens
(256 tokens/core). Each core streams the full layer weights (fp8, 33 MB)
during the layer phase and the full lm_head (fp8, 65 MB) during the head
phase; the head stays compute-bound (2.3x margin over DMA at 358 GB/s).
There are NO collectives — cores never exchange data, which removes the
AllGather straggler coupling (per-core clock-throttle variance made a
tensor-parallel head's gather completion highly variable) and the
collective bootstrap barrier (whose ring ownership blocks XBAR
transposes).

Precision: ternary weights are scaled by an exact power of two (2^-e, e~6)
so they are EXACT in fp8e4; the compensating factor (s * 2^e ~= 1) is folded
into the activation cast. Matmuls run fp16 (activations, stationary) x
fp8 (weights, moving); PSUM accumulates fp32; the residual/LN path stays
fp32. Logits are written bf16 and upcast on the host.

Scheduling:
  - The PE runs matmuls ONLY. Activation transposes go through the DMA XBAR
    (dma_start(transpose=True), fp16, SBUF->SBUF, out[p, kt, tok] =
    in[tok, kt*128 + p] — verified against CoreSim's InstDmaTransposeAnt),
    alternating between the two HWDGE queues (sync/scalar) by tile parity.
  - Software pipelining by emission order: each tile's next-layer cast is
    emitted directly after its ln_finish, so the (strict-FIFO) vector engine
    runs it before the other tile's LN chain, and the XBAR transpose lands
    during the other tile's matmuls. Layer-0 transposed inputs come from
    the host ("h0T"), so the first matmul waits only on two small DMAs.
  - Weights ride the scalar-engine DGE queue; transposes and small loads
    ride the sync queue.
  - The board power governor caps sustained near-100%-duty matmul streams
    at K=13/16 (~1.95 GHz, type-31 HAM events) with large run-to-run
    variance; minimizing total PE cycles is what matters, so transposes are
    kept off the PE even though the PE has idle slots for them.
"""

import os
import sys

import numpy as np

try:
    import concourse.bass as bass
except ImportError:  # grading container should have it on sys.path already
    sys.path.insert(0, "/opt/trn_rl_repo")
    import concourse.bass as bass

import ml_dtypes
import concourse.mybir as mybir
import concourse.tile as tile
from concourse import bacc
from concourse.bass_utils import run_bass_kernel_spmd
from contextlib import ExitStack

F32 = mybir.dt.float32
BF16 = mybir.dt.bfloat16
FP16 = mybir.dt.float16
FP8 = mybir.dt.float8e4
AX = mybir.AxisListType
OP = mybir.AluOpType
AF = mybir.ActivationFunctionType
EPS = 1e-5

# Full-size problem config (B=2, S=1024 -> 2048 tokens).
# CV: vocab chunk streamed per head step.
CFG_FULL = dict(L=8, H=2048, NTOK=2048, NC=8, TT=2, V=32000, NV=500, CH=512,
                CV=2000)


def build_nc(cfg, sigmas, head_sigma, use_gb):
    L, H, NTOK, NC, TT = cfg["L"], cfg["H"], cfg["NTOK"], cfg["NC"], cfg["TT"]
    V, NV, CH, CV = cfg["V"], cfg["NV"], cfg["CH"], cfg["CV"]
    KT = H // 128
    KH = KT // 2  # k-tiles per weight half
    NCH = H // CH
    NVC = CV // NV  # vocab sub-chunks per streamed chunk (4)
    NCHK = V // CV  # streamed head chunks (16)
    TPC = TT * 128
    assert NTOK == NC * TPC

    nc = bacc.Bacc("TRN2", target_bir_lowering=False, debug=False, num_devices=NC)
    h0 = nc.declare_dram_parameter("h0", [TT, 128, H], BF16, isOutput=False)
    h0T = nc.declare_dram_parameter("h0T", [TT, 128, KT, 128], FP16, isOutput=False)
    w_ = nc.declare_dram_parameter("w", [L, KT, 128, H], FP8, isOutput=False)
    if use_gb:
        lng = nc.declare_dram_parameter("lng", [L, H], BF16, isOutput=False)
        lnb = nc.declare_dram_parameter("lnb", [L, H], BF16, isOutput=False)
        lbias = nc.declare_dram_parameter("lbias", [L, H], BF16, isOutput=False)
        fing = nc.declare_dram_parameter("fing", [H], BF16, isOutput=False)
        finb = nc.declare_dram_parameter("finb", [H], BF16, isOutput=False)
    hw_ = nc.declare_dram_parameter("hw", [KT, 128, V], FP8, isOutput=False)
    ident_d = nc.declare_dram_parameter("ident", [128, 128], F32, isOutput=False)
    eps_d = nc.declare_dram_parameter("eps", [128, 1], F32, isOutput=False)
    out = nc.declare_dram_parameter("out", [TPC, V], BF16, isOutput=True)

    with tile.TileContext(nc) as tc:
        with ExitStack() as ctxA:
            consts = ctxA.enter_context(tc.tile_pool(name="consts", bufs=1))
            state = ctxA.enter_context(tc.tile_pool(name="state", bufs=3))
            zpool = ctxA.enter_context(tc.tile_pool(name="z", bufs=2))
            hscp = ctxA.enter_context(tc.tile_pool(name="hsc", bufs=2))
            hTp = ctxA.enter_context(tc.tile_pool(name="hT", bufs=2))
            wp = ctxA.enter_context(tc.tile_pool(name="w", bufs=8))
            hwp = ctxA.enter_context(tc.tile_pool(name="hw", bufs=2))
            outp = ctxA.enter_context(tc.tile_pool(name="outstg", bufs=4))
            gbp = None
            if use_gb:
                gbp = ctxA.enter_context(tc.tile_pool(name="gb", bufs=2))
            smp = ctxA.enter_context(tc.tile_pool(name="small", bufs=16))
            psY = ctxA.enter_context(
                tc.tile_pool(name="psY", bufs=2 * NCH, space="PSUM")
            )

            eps_t = consts.tile([128, 1], F32)
            nc.sync.dma_start(eps_t[:], eps_d[:])
            ident = consts.tile([128, 128], F32)
            nc.sync.dma_start(ident[:], ident_d[:])

            def load_w(l):
                """Layer weights in 4-ktile quarters — fine granularity lets
                each layer's first matmuls start before the whole 4.2 MB
                layer arrives (the early phase is DMA-bandwidth-starved).
                All on the scalar queue: mixing weight DMAs onto the sync
                queue alongside XBAR transposes corrupts transfers.
                """
                qs = []
                for qf in range(4):
                    wt = wp.tile([128, KT // 4, H], FP8, tag="w", name=f"w{l}_{qf}")
                    nc.scalar.dma_start(
                        wt[:],
                        w_[l, qf * (KT // 4) : (qf + 1) * (KT // 4)].rearrange(
                            "k p o -> p k o"
                        ),
                    )
                    qs.append(wt)
                return qs

            w_half0 = load_w(0)

            h_cur = []
            hT = []
            for t in range(TT):
                # layer-0 transposed input comes precomputed from the host
                ht = hTp.tile([128, KT, 128], FP16, tag="hT", name=f"hTp{t}")
                nc.sync.dma_start(ht[:], h0T[t])
                hT.append(ht)
                st = state.tile([128, H], BF16, name=f"hinit{t}", tag="state")
                h_cur.append(st)
            # residual loads: plenty of slack (first needed ~35us in)
            for t in range(TT):
                nc.scalar.dma_start(h_cur[t][:], h0[t])
            w_half1 = load_w(1)

            def cast_transpose(src_f32, scale_imm, pool, name, t):
                """h [128tok, H] f32 -> hT [128, KT, 128tok] fp16 * scale."""
                hsc = hscp.tile([128, H], FP16, tag="hsc", name=f"hsc{name}")
                nc.vector.tensor_scalar_mul(hsc[:], src_f32[:], float(scale_imm))
                dst = pool.tile([128, KT, 128], FP16, tag="hT", name=f"hT{name}")
                eng = nc.sync if t == 0 else nc.scalar
                eng.dma_start(dst[:], hsc[:], transpose=True)
                return dst

            def pe_transpose(src_f32, scale_imm, pool, name):
                """PE-based transpose via psY chunks — used for the early
                layers, when the DMA queues are bandwidth-starved."""
                dst = pool.tile([128, KT, 128], FP16, tag="hT", name=f"hT{name}")
                for j in range(NCH):
                    pc = psY.tile([128, CH], F32, tag="psY", name=f"pT{name}_{j}")
                    for u in range(CH // 128):
                        kt = j * (CH // 128) + u
                        nc.tensor.transpose(
                            pc[:, u * 128 : (u + 1) * 128],
                            src_f32[:, kt * 128 : (kt + 1) * 128],
                            ident[:],
                        )
                    nc.scalar.activation(
                        dst[:, j * (CH // 128) : (j + 1) * (CH // 128), :],
                        pc[:],
                        AF.Copy,
                        scale=float(scale_imm),
                    )
                return dst

            def ln_finish(affine_src, S_ap, SS_ap, g_t, b_t, name):
                S = smp.tile([128, 1], F32, tag="s0", name=f"S{name}")
                SS = smp.tile([128, 1], F32, tag="s1", name=f"SS{name}")
                nc.vector.tensor_reduce(S[:], S_ap, axis=AX.X, op=OP.add)
                nc.vector.tensor_reduce(SS[:], SS_ap, axis=AX.X, op=OP.add)
                negmean = smp.tile([128, 1], F32, tag="s2", name=f"nm{name}")
                nc.vector.tensor_scalar_mul(negmean[:], S[:], -1.0 / H)
                msq = smp.tile([128, 1], F32, tag="s3", name=f"msq{name}")
                nc.vector.tensor_scalar_mul(msq[:], SS[:], 1.0 / H)
                var = smp.tile([128, 1], F32, tag="s4", name=f"var{name}")
                nc.vector.tensor_tensor(var[:], negmean[:], negmean[:], OP.mult)
                nc.vector.tensor_tensor(var[:], msq[:], var[:], OP.subtract)
                std = smp.tile([128, 1], F32, tag="s5", name=f"std{name}")
                nc.scalar.activation(std[:], var[:], AF.Sqrt, bias=eps_t[:])
                rstd = smp.tile([128, 1], F32, tag="s6", name=f"rstd{name}")
                nc.vector.reciprocal(rstd[:], std[:])
                hn = state.tile([128, H], F32, tag="state", name=f"h{name}")
                nc.vector.tensor_scalar(
                    hn[:], affine_src[:], negmean[:], rstd[:], OP.add, OP.mult
                )
                if g_t is not None:
                    nc.vector.tensor_tensor(hn[:], hn[:], g_t[:], OP.mult)
                    nc.vector.tensor_tensor(hn[:], hn[:], b_t[:], OP.add)
                return hn

            fg = fb = None
            if use_gb:
                fg = gbp.tile([128, H], BF16, tag="g", name="gfin")
                nc.scalar.dma_start(fg[:], fing[None, :].to_broadcast((128, H)))
                fb = gbp.tile([128, H], BF16, tag="b", name="bfin")
                nc.scalar.dma_start(fb[:], finb[None, :].to_broadcast((128, H)))

            hTfin = [None] * TT

            def emit_final(t):
                """Final LN -> transposed head input for tile t.

                When the final affine is trivial (this input), LN of an
                LN output is the identity to ~1e-6 relative — skip it and
                transpose the layer-8 LN output directly.
                """
                h8 = h_cur[t]
                if not use_gb:
                    hTfin[t] = cast_transpose(h8, head_sigma, hTp, f"fin{t}", t)
                    return
                if t == 0:
                    fsums = smp.tile(
                        [128, 1 + NCH], F32, tag="fsums", name=f"smfin{t}"
                    )
                    nc.vector.tensor_reduce(
                        fsums[:, 0:1], h8[:], axis=AX.X, op=OP.add
                    )
                    for i in range(NCH):
                        dump = psY.tile(
                            [128, CH], F32, tag="psY", name=f"dmp{t}_{i}"
                        )
                        nc.scalar.activation(
                            dump[:],
                            h8[:, i * CH : (i + 1) * CH],
                            AF.Square,
                            accum_out=fsums[:, 1 + i : 2 + i],
                        )
                    hfin = ln_finish(
                        h8, fsums[:, 0:1], fsums[:, 1 : 1 + NCH],
                        fg, fb, f"fin{t}",
                    )
                else:
                    fsums = smp.tile([128, 2], F32, tag="fsums1", name=f"smfin{t}")
                    nc.vector.tensor_reduce(
                        fsums[:, 0:1], h8[:], axis=AX.X, op=OP.add
                    )
                    fsq = zpool.tile([128, H], F32, tag="z", name=f"fsq{t}")
                    nc.vector.tensor_tensor(fsq[:], h8[:], h8[:], OP.mult)
                    nc.vector.tensor_reduce(
                        fsums[:, 1:2], fsq[:], axis=AX.X, op=OP.add
                    )
                    hfin = ln_finish(
                        h8, fsums[:, 0:1], fsums[:, 1:2], fg, fb, f"fin{t}"
                    )
                hTfin[t] = cast_transpose(hfin, head_sigma, hTp, f"fin{t}", t)

            EARLY_PE = 3  # transposes for layers 1..3 run on the PE — the
            # DMA queues are bandwidth-starved while the weight prefetch
            # backlog drains
            pending = [None, None]
            gbt = {}
            hw_pre = []  # first head chunks, prefetched under the last layer
            for l in range(L):
                if l > 1:
                    w_half = load_w(l)
                elif l == 1:
                    w_half = w_half1
                else:
                    w_half = w_half0
                if l == L - 1:
                    for ch in range(2):
                        hwt = hwp.tile(
                            [128, KT, CV], FP8, tag="hw", name=f"hw{ch}"
                        )
                        nc.scalar.dma_start(
                            hwt[:],
                            hw_[:, :, ch * CV : (ch + 1) * CV].rearrange(
                                "k p v -> p k v"
                            ),
                        )
                        hw_pre.append(hwt)
                if use_gb:
                    g_t = gbp.tile([128, H], BF16, tag="g", name=f"g{l}")
                    nc.scalar.dma_start(g_t[:], lng[l][None, :].to_broadcast((128, H)))
                    b_t = gbp.tile([128, H], BF16, tag="b", name=f"b{l}")
                    nc.scalar.dma_start(b_t[:], lnb[l][None, :].to_broadcast((128, H)))
                    bias_t = gbp.tile([128, H], BF16, tag="bias", name=f"bias{l}")
                    nc.scalar.dma_start(
                        bias_t[:], lbias[l][None, :].to_broadcast((128, H))
                    )
                    gbt = dict(g=g_t, b=b_t, bias=bias_t)

                for t in range(TT):
                    hTt = hT[t]
                    ps = []
                    for i in range(NCH):
                        p = psY.tile([128, CH], F32, tag="psY", name=f"ps{l}_{t}_{i}")
                        ps.append(p)
                    for kt in range(KT):
                        if kt == KT // 2 and pending[1 - t] is not None:
                            pending[1 - t]()
                            pending[1 - t] = None
                        wt = w_half[kt // (KT // 4)]
                        for i in range(NCH):
                            nc.tensor.matmul(
                                ps[i][:],
                                lhsT=hTt[:, kt, :],
                                rhs=wt[:, kt % (KT // 4), i * CH : (i + 1) * CH],
                                start=(kt == 0),
                                stop=(kt == KT - 1),
                                skip_group_check=True,
                            )
                    z = zpool.tile([128, H], F32, tag="z", name=f"z{l}_{t}")
                    sums = smp.tile([128, 1 + NCH], F32, tag="sums", name=f"sm{l}_{t}")
                    resid = h_cur[t]
                    if use_gb:
                        hb = zpool.tile([128, H], F32, tag="hb", name=f"hb{l}_{t}")
                        nc.vector.tensor_tensor(hb[:], h_cur[t][:], gbt["bias"][:], OP.add)
                        resid = hb
                    for i in range(NCH):
                        nc.vector.tensor_add(
                            z[:, i * CH : (i + 1) * CH],
                            ps[i][:],
                            resid[:, i * CH : (i + 1) * CH],
                        )
                    nc.vector.tensor_reduce(sums[:, 0:1], z[:], axis=AX.X, op=OP.add)
                    for i in range(NCH):
                        nc.scalar.activation(
                            ps[i][:],
                            z[:, i * CH : (i + 1) * CH],
                            AF.Square,
                            accum_out=sums[:, 1 + i : 2 + i],
                        )
                    h_cur[t] = ln_finish(
                        z, sums[:, 0:1], sums[:, 1 : 1 + NCH],
                        gbt.get("g"), gbt.get("b"), f"{l}_{t}",
                    )
                    if l + 1 < L:
                        if l + 1 <= EARLY_PE:
                            def mk(tt, ll, src):
                                def emit():
                                    hT[tt] = pe_transpose(
                                        src, sigmas[ll + 1], hTp, f"{ll + 1}_{tt}"
                                    )
                                return emit
                            pending[t] = mk(t, l, h_cur[t])
                        else:
                            # emitted NOW: the cast sits right behind this
                            # tile's LN in the vector FIFO and the XBAR runs
                            # during the other tile's matmuls
                            hT[t] = cast_transpose(
                                h_cur[t], sigmas[l + 1], hTp, f"{l + 1}_{t}", t
                            )
                    else:
                        emit_final(t)

            # head: stream the full lm_head in CV-wide vocab chunks; each
            # chunk serves both token tiles (compute:DMA ~ 2.3:1)
            for ch in range(NCHK):
                if ch < 2:
                    hwt = hw_pre[ch]
                else:
                    hwt = hwp.tile([128, KT, CV], FP8, tag="hw", name=f"hw{ch}")
                    nc.scalar.dma_start(
                        hwt[:],
                        hw_[:, :, ch * CV : (ch + 1) * CV].rearrange("k p v -> p k v"),
                    )
                for t in range(TT):
                    pss = [
                        psY.tile([128, CH], F32, tag="psY", name=f"ph{ch}_{t}_{v}")
                        for v in range(NVC)
                    ]
                    for kt in range(KT):
                        for vi in range(NVC):
                            nc.tensor.matmul(
                                pss[vi][:, 0:NV],
                                lhsT=hTfin[t][:, kt, :],
                                rhs=hwt[:, kt, vi * NV : (vi + 1) * NV],
                                start=(kt == 0),
                                stop=(kt == KT - 1),
                                skip_group_check=True,
                            )
                    o_t = outp.tile([128, CV], BF16, tag="ostg", name=f"o{ch}_{t}")
                    for vi in range(NVC):
                        nc.scalar.copy(
                            o_t[:, vi * NV : (vi + 1) * NV], pss[vi][:, 0:NV]
                        )
                    nc.sync.dma_start(
                        out[t * 128 : (t + 1) * 128, ch * CV : (ch + 1) * CV],
                        o_t[:],
                    )

    return nc


def _ternary(wmat):
    """Exact {-1,0,1} ternary tensor + fp32 scale, matching the reference."""
    w = np.asarray(wmat, dtype=np.float32)
    s = np.mean(np.abs(w), dtype=np.float32)
    t = np.clip(np.rint(w / (s + np.float32(1e-8))), -1.0, 1.0).astype(np.float32)
    return t, float(s)


def _split_scale(s):
    """s = sigma * 2^-e with sigma ~ 1 and 2^-e exact in fp8e4."""
    e = int(np.clip(np.round(-np.log2(s)), -7, 9))
    return s * (2.0**e), e


_NC_CACHE = {}
_LAST_RESULTS = None


def kernel(**inputs):
    global _LAST_RESULTS
    cfg = CFG_FULL
    L, H, NTOK, NC, TT, V = (
        cfg["L"], cfg["H"], cfg["NTOK"], cfg["NC"], cfg["TT"], cfg["V"],
    )
    KT = H // 128
    TPC = TT * 128  # tokens per core
    BF = ml_dtypes.bfloat16
    F8 = ml_dtypes.float8_e4m3fn

    ids = np.asarray(inputs["input_ids"]).astype(np.int64).reshape(-1)
    embed = np.asarray(inputs["embed"], dtype=np.float32)
    layer_w = np.asarray(inputs["layer_w"], dtype=np.float32)
    layer_b = np.asarray(inputs["layer_b"], dtype=np.float32)
    ln_g = np.asarray(inputs["ln_g"], dtype=np.float32)
    ln_b = np.asarray(inputs["ln_b"], dtype=np.float32)
    final_g = np.asarray(inputs["final_g"], dtype=np.float32)
    final_b = np.asarray(inputs["final_b"], dtype=np.float32)
    head_w = np.asarray(inputs["head_w"], dtype=np.float32)

    use_gb = not (
        np.all(layer_b == 0.0)
        and np.all(ln_g == 1.0)
        and np.all(ln_b == 0.0)
        and np.all(final_g == 1.0)
        and np.all(final_b == 0.0)
    )

    h0_full = embed[ids]  # [NTOK, H] fp32

    sigmas = []
    wT = np.empty([L, KT, 128, H], dtype=F8)
    for l in range(L):
        t, s = _ternary(layer_w[l])
        sig, e = _split_scale(s)
        sigmas.append(sig)
        wT[l] = (
            (np.ascontiguousarray(t.T) * np.float32(2.0**-e))
            .reshape(KT, 128, H)
            .astype(F8)
        )
    th, head_scale = _ternary(head_w)
    head_sigma, e_h = _split_scale(head_scale)
    headT = (
        (np.ascontiguousarray(th.T) * np.float32(2.0**-e_h))
        .reshape(KT, 128, -1)
        .astype(F8)
    )  # [KT, 128, V]

    key = (id(cfg), tuple(sigmas), head_sigma, use_gb)
    if key not in _NC_CACHE:
        _NC_CACHE.clear()
        nc = build_nc(cfg, sigmas, head_sigma, use_gb)
        # Bacc.finalize runs the TRN2 legalization passes (1-wait-per-
        # instruction event-semaphore split, matmul->ldweights wait motion,
        # register allocation). The PJRT exec path serializes nc as-is.
        nc.finalize()
        _NC_CACHE[key] = nc
    nc = _NC_CACHE[key]

    common = {
        "w": wT,
        "hw": headT,
        "ident": np.eye(128, dtype=np.float32),
        "eps": np.full((128, 1), EPS, np.float32),
    }
    if use_gb:
        common.update(
            lng=ln_g.astype(BF),
            lnb=ln_b.astype(BF),
            lbias=layer_b.astype(BF),
            fing=final_g.astype(BF),
            finb=final_b.astype(BF),
        )
    in_maps = []
    for c in range(NC):
        h0c = np.ascontiguousarray(
            h0_full[c * TPC : (c + 1) * TPC].reshape(TT, 128, H)
        )
        # layer-0 transposed+scaled input, in XBAR block layout
        h0t = np.ascontiguousarray(
            (h0c * np.float32(sigmas[0]))
            .transpose(0, 2, 1)  # [TT, H, 128tok]
            .reshape(TT, KT, 128, 128)
            .transpose(0, 2, 1, 3)  # [TT, 128p, KT, 128tok]
        ).astype(np.float16)
        in_maps.append(dict(common, h0=h0c.astype(BF), h0T=h0t))

    trace = bool(int(os.environ.get("TRIKERNEL_TRACE", "0")))
    res = run_bass_kernel_spmd(nc, in_maps, core_ids=list(range(NC)), trace=trace)
    _LAST_RESULTS = res

    full = np.concatenate(
        [np.asarray(res.results[c]["out"]) for c in range(NC)], axis=0
    )  # [NTOK, V] bf16
    return full.reshape(2, 1024, 32000).astype(np.float32)


# revision 56
# speedup vs baseline: 1.0408x; 1.0020x over previous
"""Trainium2 Bass kernel: 8-layer ternary (BitNet-1.58) dense transformer.

Model (per reference):
    h = embed[input_ids]                                  # (B=2, S=1024, H=2048)
    8x: y = h @ ternary(W_l)^T + b_l ; h = LN(y + h)*g+b  # H=2048
    h = LN(h)*final_g + final_b
    logits = h @ ternary(head_W)^T                        # (B, S, V=32000)

Sharding over 8 NeuronCores: fully data-parallel over the 2048 tokens
(256 tokens/core). Each core streams the full layer weights (fp8, 33 MB)
during the layer phase and the full lm_head (fp8, 65 MB) during the head
phase; the head stays compute-bound (2.3x margin over DMA at 358 GB/s).
There are NO collectives — cores never exchange data, which removes the
AllGather straggler coupling (per-core clock-throttle variance made a
tensor-parallel head's gather completion highly variable) and the
collective bootstrap barrier (whose ring ownership blocks XBAR
transposes).

Precision: ternary weights are scaled by an exact power of two (2^-e, e~6)
so they are EXACT in fp8e4; the compensating factor (s * 2^e ~= 1) is folded
into the activation cast. Matmuls run fp16 (activations, stationary) x
fp8 (weights, moving); PSUM accumulates fp32; the residual/LN path stays
fp32. Logits are written bf16 and upcast on the host.

Scheduling:
  - The PE runs matmuls ONLY. Activation transposes go through the DMA XBAR
    (dma_start(transpose=True), fp16, SBUF->SBUF, out[p, kt, tok] =
    in[tok, kt*128 + p] — verified against CoreSim's InstDmaTransposeAnt),
    alternating between the two HWDGE queues (sync/scalar) by tile parity.
  - Software pipelining by emission order: each tile's next-layer cast is
    emitted directly after its ln_finish, so the (strict-FIFO) vector engine
    runs it before the other tile's LN chain, and the XBAR transpose lands
    during the other tile's matmuls. Layer-0 transposed inputs come from
    the host ("h0T"), so the first matmul waits only on two small DMAs.
  - Weights ride the scalar-engine DGE queue; transposes and small loads
    ride the sync queue.
  - The board power governor caps sustained near-100%-duty matmul streams
    at K=13/16 (~1.95 GHz, type-31 HAM events) with large run-to-run
    variance; minimizing total PE cycles is what matters, so transposes are
    kept off the PE even though the PE has idle slots for them.
"""

import os
import sys

import numpy as np

try:
    import concourse.bass as bass
except ImportError:  # grading container should have it on sys.path already
    sys.path.insert(0, "/opt/trn_rl_repo")
    import concourse.bass as bass

import ml_dtypes
import concourse.mybir as mybir
import concourse.tile as tile
from concourse import bacc
from concourse.bass_utils import run_bass_kernel_spmd
from contextlib import ExitStack

F32 = mybir.dt.float32
BF16 = mybir.dt.bfloat16
FP16 = mybir.dt.float16
FP8 = mybir.dt.float8e4
AX = mybir.AxisListType
OP = mybir.AluOpType
AF = mybir.ActivationFunctionType
EPS = 1e-5

# Full-size problem config (B=2, S=1024 -> 2048 tokens).
# CV: vocab chunk streamed per head step.
CFG_FULL = dict(L=8, H=2048, NTOK=2048, NC=8, TT=2, V=32000, NV=500, CH=512,
                CV=2000)


def build_nc(cfg, sigmas, head_sigma, use_gb):
    L, H, NTOK, NC, TT = cfg["L"], cfg["H"], cfg["NTOK"], cfg["NC"], cfg["TT"]
    V, NV, CH, CV = cfg["V"], cfg["NV"], cfg["CH"], cfg["CV"]
    KT = H // 128
    KH = KT // 2  # k-tiles per weight half
    NCH = H // CH
    NVC = CV // NV  # vocab sub-chunks per streamed chunk (4)
    NCHK = V // CV  # streamed head chunks (16)
    TPC = TT * 128
    assert NTOK == NC * TPC

    nc = bacc.Bacc("TRN2", target_bir_lowering=False, debug=False, num_devices=NC)
    h0 = nc.declare_dram_parameter("h0", [TT, 128, H], BF16, isOutput=False)
    h0T = nc.declare_dram_parameter("h0T", [TT, 128, KT, 128], FP16, isOutput=False)
    w_ = nc.declare_dram_parameter("w", [L, KT, 128, H], FP8, isOutput=False)
    if use_gb:
        lng = nc.declare_dram_parameter("lng", [L, H], BF16, isOutput=False)
        lnb = nc.declare_dram_parameter("lnb", [L, H], BF16, isOutput=False)
        lbias = nc.declare_dram_parameter("lbias", [L, H], BF16, isOutput=False)
        fing = nc.declare_dram_parameter("fing", [H], BF16, isOutput=False)
        finb = nc.declare_dram_parameter("finb", [H], BF16, isOutput=False)
    hw_ = nc.declare_dram_parameter("hw", [KT, 128, V], FP8, isOutput=False)
    ident_d = nc.declare_dram_parameter("ident", [128, 128], F32, isOutput=False)
    eps_d = nc.declare_dram_parameter("eps", [128, 1], F32, isOutput=False)
    out = nc.declare_dram_parameter("out", [TPC, V], BF16, isOutput=True)

    with tile.TileContext(nc) as tc:
        with ExitStack() as ctxA:
            consts = ctxA.enter_context(tc.tile_pool(name="consts", bufs=1))
            state = ctxA.enter_context(tc.tile_pool(name="state", bufs=3))
            zpool = ctxA.enter_context(tc.tile_pool(name="z", bufs=2))
            hscp = ctxA.enter_context(tc.tile_pool(name="hsc", bufs=2))
            hTp = ctxA.enter_context(tc.tile_pool(name="hT", bufs=2))
            wp = ctxA.enter_context(tc.tile_pool(name="w", bufs=8))
            hwp = ctxA.enter_context(tc.tile_pool(name="hw", bufs=2))
            outp = ctxA.enter_context(tc.tile_pool(name="outstg", bufs=4))
            gbp = None
            if use_gb:
                gbp = ctxA.enter_context(tc.tile_pool(name="gb", bufs=2))
            smp = ctxA.enter_context(tc.tile_pool(name="small", bufs=16))
            psY = ctxA.enter_context(
                tc.tile_pool(name="psY", bufs=2 * NCH, space="PSUM")
            )

            eps_t = consts.tile([128, 1], F32)
            nc.sync.dma_start(eps_t[:], eps_d[:])
            ident = consts.tile([128, 128], F32)
            nc.sync.dma_start(ident[:], ident_d[:])

            def load_w(l):
                """Layer weights in 4-ktile quarters — fine granularity lets
                each layer's first matmuls start before the whole 4.2 MB
                layer arrives (the early phase is DMA-bandwidth-starved).
                All on the scalar queue: mixing weight DMAs onto the sync
                queue alongside XBAR transposes corrupts transfers.
                """
                qs = []
                for qf in range(4):
                    wt = wp.tile([128, KT // 4, H], FP8, tag="w", name=f"w{l}_{qf}")
                    nc.scalar.dma_start(
                        wt[:],
                        w_[l, qf * (KT // 4) : (qf + 1) * (KT // 4)].rearrange(
                            "k p o -> p k o"
                        ),
                    )
                    qs.append(wt)
                return qs

            w_half0 = load_w(0)

            h_cur = []
            hT = []
            for t in range(TT):
                # layer-0 transposed input comes precomputed from the host
                ht = hTp.tile([128, KT, 128], FP16, tag="hT", name=f"hTp{t}")
                nc.sync.dma_start(ht[:], h0T[t])
                hT.append(ht)
                st = state.tile([128, H], BF16, name=f"hinit{t}", tag="state")
                h_cur.append(st)
            # residual loads: plenty of slack (first needed ~35us in)
            for t in range(TT):
                nc.scalar.dma_start(h_cur[t][:], h0[t])
            w_half1 = load_w(1)

            def cast_transpose(src_f32, scale_imm, pool, name, t):
                """h [128tok, H] f32 -> hT [128, KT, 128tok] fp16 * scale."""
                hsc = hscp.tile([128, H], FP16, tag="hsc", name=f"hsc{name}")
                nc.vector.tensor_scalar_mul(hsc[:], src_f32[:], float(scale_imm))
                dst = pool.tile([128, KT, 128], FP16, tag="hT", name=f"hT{name}")
                # layer XBARs all ride sync (no weight traffic there, so no
                # ring-corruption hazard, and they stop delaying the weight
                # quarter issues queued on scalar). Tile 1's FINAL transpose
                # stays on scalar: it completes early in the head phase and
                # must not overlap the head-weight DMAs starting on sync.
                eng = nc.scalar if name == "fin1" else nc.sync
                eng.dma_start(dst[:], hsc[:], transpose=True)
                return dst

            def pe_transpose(src_f32, scale_imm, pool, name):
                """PE-based transpose via psY chunks — used for the early
                layers, when the DMA queues are bandwidth-starved."""
                dst = pool.tile([128, KT, 128], FP16, tag="hT", name=f"hT{name}")
                for j in range(NCH):
                    pc = psY.tile([128, CH], F32, tag="psY", name=f"pT{name}_{j}")
                    for u in range(CH // 128):
                        kt = j * (CH // 128) + u
                        nc.tensor.transpose(
                            pc[:, u * 128 : (u + 1) * 128],
                            src_f32[:, kt * 128 : (kt + 1) * 128],
                            ident[:],
                        )
                    nc.scalar.activation(
                        dst[:, j * (CH // 128) : (j + 1) * (CH // 128), :],
                        pc[:],
                        AF.Copy,
                        scale=float(scale_imm),
                    )
                return dst

            def ln_finish(affine_src, S_ap, SS_ap, g_t, b_t, name):
                S = smp.tile([128, 1], F32, tag="s0", name=f"S{name}")
                SS = smp.tile([128, 1], F32, tag="s1", name=f"SS{name}")
                nc.vector.tensor_reduce(S[:], S_ap, axis=AX.X, op=OP.add)
                nc.vector.tensor_reduce(SS[:], SS_ap, axis=AX.X, op=OP.add)
                negmean = smp.tile([128, 1], F32, tag="s2", name=f"nm{name}")
                nc.vector.tensor_scalar_mul(negmean[:], S[:], -1.0 / H)
                msq = smp.tile([128, 1], F32, tag="s3", name=f"msq{name}")
                nc.vector.tensor_scalar_mul(msq[:], SS[:], 1.0 / H)
                var = smp.tile([128, 1], F32, tag="s4", name=f"var{name}")
                nc.vector.tensor_tensor(var[:], negmean[:], negmean[:], OP.mult)
                nc.vector.tensor_tensor(var[:], msq[:], var[:], OP.subtract)
                std = smp.tile([128, 1], F32, tag="s5", name=f"std{name}")
                nc.scalar.activation(std[:], var[:], AF.Sqrt, bias=eps_t[:])
                rstd = smp.tile([128, 1], F32, tag="s6", name=f"rstd{name}")
                nc.vector.reciprocal(rstd[:], std[:])
                hn = state.tile([128, H], F32, tag="state", name=f"h{name}")
                nc.vector.tensor_scalar(
                    hn[:], affine_src[:], negmean[:], rstd[:], OP.add, OP.mult
                )
                if g_t is not None:
                    nc.vector.tensor_tensor(hn[:], hn[:], g_t[:], OP.mult)
                    nc.vector.tensor_tensor(hn[:], hn[:], b_t[:], OP.add)
                return hn

            fg = fb = None
            if use_gb:
                fg = gbp.tile([128, H], BF16, tag="g", name="gfin")
                nc.scalar.dma_start(fg[:], fing[None, :].to_broadcast((128, H)))
                fb = gbp.tile([128, H], BF16, tag="b", name="bfin")
                nc.scalar.dma_start(fb[:], finb[None, :].to_broadcast((128, H)))

            hTfin = [None] * TT

            def emit_final(t):
                """Final LN -> transposed head input for tile t.

                When the final affine is trivial (this input), LN of an
                LN output is the identity to ~1e-6 relative — skip it and
                transpose the layer-8 LN output directly.
                """
                h8 = h_cur[t]
                if not use_gb:
                    hTfin[t] = cast_transpose(h8, head_sigma, hTp, f"fin{t}", t)
                    return
                if t == 0:
                    fsums = smp.tile(
                        [128, 1 + NCH], F32, tag="fsums", name=f"smfin{t}"
                    )
                    nc.vector.tensor_reduce(
                        fsums[:, 0:1], h8[:], axis=AX.X, op=OP.add
                    )
                    for i in range(NCH):
                        dump = psY.tile(
                            [128, CH], F32, tag="psY", name=f"dmp{t}_{i}"
                        )
                        nc.scalar.activation(
                            dump[:],
                            h8[:, i * CH : (i + 1) * CH],
                            AF.Square,
                            accum_out=fsums[:, 1 + i : 2 + i],
                        )
                    hfin = ln_finish(
                        h8, fsums[:, 0:1], fsums[:, 1 : 1 + NCH],
                        fg, fb, f"fin{t}",
                    )
                else:
                    fsums = smp.tile([128, 2], F32, tag="fsums1", name=f"smfin{t}")
                    nc.vector.tensor_reduce(
                        fsums[:, 0:1], h8[:], axis=AX.X, op=OP.add
                    )
                    fsq = zpool.tile([128, H], F32, tag="z", name=f"fsq{t}")
                    nc.vector.tensor_tensor(fsq[:], h8[:], h8[:], OP.mult)
                    nc.vector.tensor_reduce(
                        fsums[:, 1:2], fsq[:], axis=AX.X, op=OP.add
                    )
                    hfin = ln_finish(
                        h8, fsums[:, 0:1], fsums[:, 1:2], fg, fb, f"fin{t}"
                    )
                hTfin[t] = cast_transpose(hfin, head_sigma, hTp, f"fin{t}", t)

            EARLY_PE = 3  # transposes for layers 1..3 run on the PE — the
            # DMA queues are bandwidth-starved while the weight prefetch
            # backlog drains
            pending = [None, None]
            gbt = {}
            hw_pre = []  # first head chunks, prefetched under the last layer
            for l in range(L):
                if l > 1:
                    w_half = load_w(l)
                elif l == 1:
                    w_half = w_half1
                else:
                    w_half = w_half0
                if l == L - 1:
                    for ch in range(2):
                        hwt = hwp.tile(
                            [128, KT, CV], FP8, tag="hw", name=f"hw{ch}"
                        )
                        nc.scalar.dma_start(
                            hwt[:],
                            hw_[:, :, ch * CV : (ch + 1) * CV].rearrange(
                                "k p v -> p k v"
                            ),
                        )
                        hw_pre.append(hwt)
                if use_gb:
                    g_t = gbp.tile([128, H], BF16, tag="g", name=f"g{l}")
                    nc.scalar.dma_start(g_t[:], lng[l][None, :].to_broadcast((128, H)))
                    b_t = gbp.tile([128, H], BF16, tag="b", name=f"b{l}")
                    nc.scalar.dma_start(b_t[:], lnb[l][None, :].to_broadcast((128, H)))
                    bias_t = gbp.tile([128, H], BF16, tag="bias", name=f"bias{l}")
                    nc.scalar.dma_start(
                        bias_t[:], lbias[l][None, :].to_broadcast((128, H))
                    )
                    gbt = dict(g=g_t, b=b_t, bias=bias_t)

                for t in range(TT):
                    hTt = hT[t]
                    ps = []
                    for i in range(NCH):
                        p = psY.tile([128, CH], F32, tag="psY", name=f"ps{l}_{t}_{i}")
                        ps.append(p)
                    for kt in range(KT):
                        if kt == KT // 2 and pending[1 - t] is not None:
                            pending[1 - t]()
                            pending[1 - t] = None
                        wt = w_half[kt // (KT // 4)]
                        for i in range(NCH):
                            nc.tensor.matmul(
                                ps[i][:],
                                lhsT=hTt[:, kt, :],
                                rhs=wt[:, kt % (KT // 4), i * CH : (i + 1) * CH],
                                start=(kt == 0),
                                stop=(kt == KT - 1),
                                skip_group_check=True,
                            )
                    z = zpool.tile([128, H], F32, tag="z", name=f"z{l}_{t}")
                    sums = smp.tile([128, 1 + NCH], F32, tag="sums", name=f"sm{l}_{t}")
                    resid = h_cur[t]
                    if use_gb:
                        hb = zpool.tile([128, H], F32, tag="hb", name=f"hb{l}_{t}")
                        nc.vector.tensor_tensor(hb[:], h_cur[t][:], gbt["bias"][:], OP.add)
                        resid = hb
                    for i in range(NCH):
                        nc.vector.tensor_add(
                            z[:, i * CH : (i + 1) * CH],
                            ps[i][:],
                            resid[:, i * CH : (i + 1) * CH],
                        )
                    nc.vector.tensor_reduce(sums[:, 0:1], z[:], axis=AX.X, op=OP.add)
                    for i in range(NCH):
                        nc.scalar.activation(
                            ps[i][:],
                            z[:, i * CH : (i + 1) * CH],
                            AF.Square,
                            accum_out=sums[:, 1 + i : 2 + i],
                        )
                    h_cur[t] = ln_finish(
                        z, sums[:, 0:1], sums[:, 1 : 1 + NCH],
                        gbt.get("g"), gbt.get("b"), f"{l}_{t}",
                    )
                    if l + 1 < L:
                        if l + 1 <= EARLY_PE:
                            def mk(tt, ll, src):
                                def emit():
                                    hT[tt] = pe_transpose(
                                        src, sigmas[ll + 1], hTp, f"{ll + 1}_{tt}"
                                    )
                                return emit
                            pending[t] = mk(t, l, h_cur[t])
                        else:
                            # emitted NOW: the cast sits right behind this
                            # tile's LN in the vector FIFO and the XBAR runs
                            # during the other tile's matmuls
                            hT[t] = cast_transpose(
                                h_cur[t], sigmas[l + 1], hTp, f"{l + 1}_{t}", t
                            )
                    else:
                        emit_final(t)

            # head: stream the full lm_head in CV-wide vocab chunks; each
            # chunk serves both token tiles (compute:DMA ~ 2.3:1)
            for ch in range(NCHK):
                if ch < 2:
                    hwt = hw_pre[ch]
                else:
                    # head weights stream on the otherwise-idle sync queue so
                    # their issue never queues behind the PSUM->staging
                    # copies (which block awaiting matmul completion)
                    hwt = hwp.tile([128, KT, CV], FP8, tag="hw", name=f"hw{ch}")
                    nc.sync.dma_start(
                        hwt[:],
                        hw_[:, :, ch * CV : (ch + 1) * CV].rearrange("k p v -> p k v"),
                    )
                for t in range(TT):
                    pss = [
                        psY.tile([128, CH], F32, tag="psY", name=f"ph{ch}_{t}_{v}")
                        for v in range(NVC)
                    ]
                    for kt in range(KT):
                        for vi in range(NVC):
                            nc.tensor.matmul(
                                pss[vi][:, 0:NV],
                                lhsT=hTfin[t][:, kt, :],
                                rhs=hwt[:, kt, vi * NV : (vi + 1) * NV],
                                start=(kt == 0),
                                stop=(kt == KT - 1),
                                skip_group_check=True,
                            )
                    o_t = outp.tile([128, CV], BF16, tag="ostg", name=f"o{ch}_{t}")
                    for vi in range(NVC):
                        nc.scalar.copy(
                            o_t[:, vi * NV : (vi + 1) * NV], pss[vi][:, 0:NV]
                        )
                    # out rides scalar, directly behind its own copies
                    nc.scalar.dma_start(
                        out[t * 128 : (t + 1) * 128, ch * CV : (ch + 1) * CV],
                        o_t[:],
                    )

    return nc


def _ternary(wmat):
    """Exact {-1,0,1} ternary tensor + fp32 scale, matching the reference."""
    w = np.asarray(wmat, dtype=np.float32)
    s = np.mean(np.abs(w), dtype=np.float32)
    t = np.clip(np.rint(w / (s + np.float32(1e-8))), -1.0, 1.0).astype(np.float32)
    return t, float(s)


def _split_scale(s):
    """s = sigma * 2^-e with sigma ~ 1 and 2^-e exact in fp8e4."""
    e = int(np.clip(np.round(-np.log2(s)), -7, 9))
    return s * (2.0**e), e


_NC_CACHE = {}
_LAST_RESULTS = None


def kernel(**inputs):
    global _LAST_RESULTS
    cfg = CFG_FULL
    L, H, NTOK, NC, TT, V = (
        cfg["L"], cfg["H"], cfg["NTOK"], cfg["NC"], cfg["TT"], cfg["V"],
    )
    KT = H // 128
    TPC = TT * 128  # tokens per core
    BF = ml_dtypes.bfloat16
    F8 = ml_dtypes.float8_e4m3fn

    ids = np.asarray(inputs["input_ids"]).astype(np.int64).reshape(-1)
    embed = np.asarray(inputs["embed"], dtype=np.float32)
    layer_w = np.asarray(inputs["layer_w"], dtype=np.float32)
    layer_b = np.asarray(inputs["layer_b"], dtype=np.float32)
    ln_g = np.asarray(inputs["ln_g"], dtype=np.float32)
    ln_b = np.asarray(inputs["ln_b"], dtype=np.float32)
    final_g = np.asarray(inputs["final_g"], dtype=np.float32)
    final_b = np.asarray(inputs["final_b"], dtype=np.float32)
    head_w = np.asarray(inputs["head_w"], dtype=np.float32)

    use_gb = not (
        np.all(layer_b == 0.0)
        and np.all(ln_g == 1.0)
        and np.all(ln_b == 0.0)
        and np.all(final_g == 1.0)
        and np.all(final_b == 0.0)
    )

    h0_full = embed[ids]  # [NTOK, H] fp32

    sigmas = []
    wT = np.empty([L, KT, 128, H], dtype=F8)
    for l in range(L):
        t, s = _ternary(layer_w[l])
        sig, e = _split_scale(s)
        sigmas.append(sig)
        wT[l] = (
            (np.ascontiguousarray(t.T) * np.float32(2.0**-e))
            .reshape(KT, 128, H)
            .astype(F8)
        )
    th, head_scale = _ternary(head_w)
    head_sigma, e_h = _split_scale(head_scale)
    headT = (
        (np.ascontiguousarray(th.T) * np.float32(2.0**-e_h))
        .reshape(KT, 128, -1)
        .astype(F8)
    )  # [KT, 128, V]

    key = (id(cfg), tuple(sigmas), head_sigma, use_gb)
    if key not in _NC_CACHE:
        _NC_CACHE.clear()
        nc = build_nc(cfg, sigmas, head_sigma, use_gb)
        # Bacc.finalize runs the TRN2 legalization passes (1-wait-per-
        # instruction event-semaphore split, matmul->ldweights wait motion,
        # register allocation). The PJRT exec path serializes nc as-is.
        nc.finalize()
        _NC_CACHE[key] = nc
    nc = _NC_CACHE[key]

    common = {
        "w": wT,
        "hw": headT,
        "ident": np.eye(128, dtype=np.float32),
        "eps": np.full((128, 1), EPS, np.float32),
    }
    if use_gb:
        common.update(
            lng=ln_g.astype(BF),
            lnb=ln_b.astype(BF),
            lbias=layer_b.astype(BF),
            fing=final_g.astype(BF),
            finb=final_b.astype(BF),
        )
    in_maps = []
    for c in range(NC):
        h0c = np.ascontiguousarray(
            h0_full[c * TPC : (c + 1) * TPC].reshape(TT, 128, H)
        )
        # layer-0 transposed+scaled input, in XBAR block layout
        h0t = np.ascontiguousarray(
            (h0c * np.float32(sigmas[0]))
            .transpose(0, 2, 1)  # [TT, H, 128tok]
            .reshape(TT, KT, 128, 128)
            .transpose(0, 2, 1, 3)  # [TT, 128p, KT, 128tok]
        ).astype(np.float16)
        in_maps.append(dict(common, h0=h0c.astype(BF), h0T=h0t))

    trace = bool(int(os.environ.get("TRIKERNEL_TRACE", "0")))
    res = run_bass_kernel_spmd(nc, in_maps, core_ids=list(range(NC)), trace=trace)
    _LAST_RESULTS = res

    full = np.concatenate(
        [np.asarray(res.results[c]["out"]) for c in range(NC)], axis=0
    )  # [NTOK, V] bf16
    return full.reshape(2, 1024, 32000).astype(np.float32)


# revision 58
# speedup vs baseline: 1.0410x; 1.0002x over previous
"""Trainium2 Bass kernel: 8-layer ternary (BitNet-1.58) dense transformer.

Model (per reference):
    h = embed[input_ids]                                  # (B=2, S=1024, H=2048)
    8x: y = h @ ternary(W_l)^T + b_l ; h = LN(y + h)*g+b  # H=2048
    h = LN(h)*final_g + final_b
    logits = h @ ternary(head_W)^T                        # (B, S, V=32000)

Sharding over 8 NeuronCores: fully data-parallel over the 2048 tokens
(256 tokens/core). Each core streams the full layer weights (fp8, 33 MB)
during the layer phase and the full lm_head (fp8, 65 MB) during the head
phase; the head stays compute-bound (2.3x margin over DMA at 358 GB/s).
There are NO collectives — cores never exchange data, which removes the
AllGather straggler coupling (per-core clock-throttle variance made a
tensor-parallel head's gather completion highly variable) and the
collective bootstrap barrier (whose ring ownership blocks XBAR
transposes).

Precision: ternary weights are scaled by an exact power of two (2^-e, e~6)
so they are EXACT in fp8e4; the compensating factor (s * 2^e ~= 1) is folded
into the activation cast. Matmuls run fp16 (activations, stationary) x
fp8 (weights, moving); PSUM accumulates fp32; the residual/LN path stays
fp32. Logits are written bf16 and upcast on the host.

Scheduling:
  - The PE runs matmuls ONLY. Activation transposes go through the DMA XBAR
    (dma_start(transpose=True), fp16, SBUF->SBUF, out[p, kt, tok] =
    in[tok, kt*128 + p] — verified against CoreSim's InstDmaTransposeAnt),
    alternating between the two HWDGE queues (sync/scalar) by tile parity.
  - Software pipelining by emission order: each tile's next-layer cast is
    emitted directly after its ln_finish, so the (strict-FIFO) vector engine
    runs it before the other tile's LN chain, and the XBAR transpose lands
    during the other tile's matmuls. Layer-0 transposed inputs come from
    the host ("h0T"), so the first matmul waits only on two small DMAs.
  - Weights ride the scalar-engine DGE queue; transposes and small loads
    ride the sync queue.
  - The board power governor caps sustained near-100%-duty matmul streams
    at K=13/16 (~1.95 GHz, type-31 HAM events) with large run-to-run
    variance; minimizing total PE cycles is what matters, so transposes are
    kept off the PE even though the PE has idle slots for them.
"""

import os
import sys

import numpy as np

try:
    import concourse.bass as bass
except ImportError:  # grading container should have it on sys.path already
    sys.path.insert(0, "/opt/trn_rl_repo")
    import concourse.bass as bass

import ml_dtypes
import concourse.mybir as mybir
import concourse.tile as tile
from concourse import bacc
from concourse.bass_utils import run_bass_kernel_spmd
from contextlib import ExitStack

F32 = mybir.dt.float32
BF16 = mybir.dt.bfloat16
FP16 = mybir.dt.float16
FP8 = mybir.dt.float8e4
AX = mybir.AxisListType
OP = mybir.AluOpType
AF = mybir.ActivationFunctionType
EPS = 1e-5

# Full-size problem config (B=2, S=1024 -> 2048 tokens).
# CV: vocab chunk streamed per head step.
CFG_FULL = dict(L=8, H=2048, NTOK=2048, NC=8, TT=2, V=32000, NV=500, CH=512,
                CV=2000)


def build_nc(cfg, sigmas, head_sigma, use_gb):
    L, H, NTOK, NC, TT = cfg["L"], cfg["H"], cfg["NTOK"], cfg["NC"], cfg["TT"]
    V, NV, CH, CV = cfg["V"], cfg["NV"], cfg["CH"], cfg["CV"]
    KT = H // 128
    KH = KT // 2  # k-tiles per weight half
    NCH = H // CH
    NVC = CV // NV  # vocab sub-chunks per streamed chunk (4)
    NCHK = V // CV  # streamed head chunks (16)
    TPC = TT * 128
    assert NTOK == NC * TPC

    nc = bacc.Bacc("TRN2", target_bir_lowering=False, debug=False, num_devices=NC)
    h0 = nc.declare_dram_parameter("h0", [TT, 128, H], BF16, isOutput=False)
    h0T = nc.declare_dram_parameter("h0T", [TT, 128, KT, 128], FP16, isOutput=False)
    w_ = nc.declare_dram_parameter("w", [L, KT, 128, H], FP8, isOutput=False)
    if use_gb:
        lng = nc.declare_dram_parameter("lng", [L, H], BF16, isOutput=False)
        lnb = nc.declare_dram_parameter("lnb", [L, H], BF16, isOutput=False)
        lbias = nc.declare_dram_parameter("lbias", [L, H], BF16, isOutput=False)
        fing = nc.declare_dram_parameter("fing", [H], BF16, isOutput=False)
        finb = nc.declare_dram_parameter("finb", [H], BF16, isOutput=False)
    hw_ = nc.declare_dram_parameter("hw", [KT, 128, V], FP8, isOutput=False)
    ident_d = nc.declare_dram_parameter("ident", [128, 128], F32, isOutput=False)
    eps_d = nc.declare_dram_parameter("eps", [128, 1], F32, isOutput=False)
    out = nc.declare_dram_parameter("out", [TPC, V], BF16, isOutput=True)

    with tile.TileContext(nc) as tc:
        with ExitStack() as ctxA:
            consts = ctxA.enter_context(tc.tile_pool(name="consts", bufs=1))
            state = ctxA.enter_context(tc.tile_pool(name="state", bufs=3))
            zpool = ctxA.enter_context(tc.tile_pool(name="z", bufs=2))
            hscp = ctxA.enter_context(tc.tile_pool(name="hsc", bufs=2))
            hTp = ctxA.enter_context(tc.tile_pool(name="hT", bufs=2))
            wp = ctxA.enter_context(tc.tile_pool(name="w", bufs=8))
            hwp = ctxA.enter_context(tc.tile_pool(name="hw", bufs=2))
            outp = ctxA.enter_context(tc.tile_pool(name="outstg", bufs=4))
            gbp = None
            if use_gb:
                gbp = ctxA.enter_context(tc.tile_pool(name="gb", bufs=2))
            smp = ctxA.enter_context(tc.tile_pool(name="small", bufs=16))
            psY = ctxA.enter_context(
                tc.tile_pool(name="psY", bufs=2 * NCH, space="PSUM")
            )

            eps_t = consts.tile([128, 1], F32)
            nc.sync.dma_start(eps_t[:], eps_d[:])
            ident = consts.tile([128, 128], F32)
            nc.sync.dma_start(ident[:], ident_d[:])

            def load_w(l):
                """Layer weights in 4-ktile quarters — fine granularity lets
                each layer's first matmuls start before the whole 4.2 MB
                layer arrives (the early phase is DMA-bandwidth-starved).
                All on the scalar queue: mixing weight DMAs onto the sync
                queue alongside XBAR transposes corrupts transfers.
                """
                qs = []
                for qf in range(4):
                    wt = wp.tile([128, KT // 4, H], FP8, tag="w", name=f"w{l}_{qf}")
                    nc.scalar.dma_start(
                        wt[:],
                        w_[l, qf * (KT // 4) : (qf + 1) * (KT // 4)].rearrange(
                            "k p o -> p k o"
                        ),
                    )
                    qs.append(wt)
                return qs

            w_half0 = load_w(0)

            h_cur = []
            hT = []
            for t in range(TT):
                # layer-0 transposed input comes precomputed from the host
                ht = hTp.tile([128, KT, 128], FP16, tag="hT", name=f"hTp{t}")
                nc.sync.dma_start(ht[:], h0T[t])
                hT.append(ht)
                st = state.tile([128, H], BF16, name=f"hinit{t}", tag="state")
                h_cur.append(st)
            # residual loads: plenty of slack (first needed ~35us in)
            for t in range(TT):
                nc.scalar.dma_start(h_cur[t][:], h0[t])
            w_half1 = load_w(1)

            def cast_transpose(src_f32, scale_imm, pool, name, t):
                """h [128tok, H] f32 -> hT [128, KT, 128tok] fp16 * scale."""
                hsc = hscp.tile([128, H], FP16, tag="hsc", name=f"hsc{name}")
                nc.vector.tensor_scalar_mul(hsc[:], src_f32[:], float(scale_imm))
                dst = pool.tile([128, KT, 128], FP16, tag="hT", name=f"hT{name}")
                # layer XBARs all ride sync (no weight traffic there, so no
                # ring-corruption hazard, and they stop delaying the weight
                # quarter issues queued on scalar). Tile 1's FINAL transpose
                # stays on scalar: it completes early in the head phase and
                # must not overlap the head-weight DMAs starting on sync.
                eng = nc.scalar if name == "fin1" else nc.sync
                eng.dma_start(dst[:], hsc[:], transpose=True)
                return dst

            def pe_transpose(src_f32, scale_imm, pool, name):
                """PE-based transpose via psY chunks — used for the early
                layers, when the DMA queues are bandwidth-starved."""
                dst = pool.tile([128, KT, 128], FP16, tag="hT", name=f"hT{name}")
                for j in range(NCH):
                    pc = psY.tile([128, CH], F32, tag="psY", name=f"pT{name}_{j}")
                    for u in range(CH // 128):
                        kt = j * (CH // 128) + u
                        nc.tensor.transpose(
                            pc[:, u * 128 : (u + 1) * 128],
                            src_f32[:, kt * 128 : (kt + 1) * 128],
                            ident[:],
                        )
                    nc.scalar.activation(
                        dst[:, j * (CH // 128) : (j + 1) * (CH // 128), :],
                        pc[:],
                        AF.Copy,
                        scale=float(scale_imm),
                    )
                return dst

            def ln_finish(affine_src, S_ap, SS_ap, g_t, b_t, name):
                S = smp.tile([128, 1], F32, tag="s0", name=f"S{name}")
                SS = smp.tile([128, 1], F32, tag="s1", name=f"SS{name}")
                nc.vector.tensor_reduce(S[:], S_ap, axis=AX.X, op=OP.add)
                nc.vector.tensor_reduce(SS[:], SS_ap, axis=AX.X, op=OP.add)
                negmean = smp.tile([128, 1], F32, tag="s2", name=f"nm{name}")
                nc.vector.tensor_scalar_mul(negmean[:], S[:], -1.0 / H)
                msq = smp.tile([128, 1], F32, tag="s3", name=f"msq{name}")
                nc.vector.tensor_scalar_mul(msq[:], SS[:], 1.0 / H)
                var = smp.tile([128, 1], F32, tag="s4", name=f"var{name}")
                nc.vector.tensor_tensor(var[:], negmean[:], negmean[:], OP.mult)
                nc.vector.tensor_tensor(var[:], msq[:], var[:], OP.subtract)
                std = smp.tile([128, 1], F32, tag="s5", name=f"std{name}")
                nc.scalar.activation(std[:], var[:], AF.Sqrt, bias=eps_t[:])
                rstd = smp.tile([128, 1], F32, tag="s6", name=f"rstd{name}")
                nc.vector.reciprocal(rstd[:], std[:])
                hn = state.tile([128, H], F32, tag="state", name=f"h{name}")
                nc.vector.tensor_scalar(
                    hn[:], affine_src[:], negmean[:], rstd[:], OP.add, OP.mult
                )
                if g_t is not None:
                    nc.vector.tensor_tensor(hn[:], hn[:], g_t[:], OP.mult)
                    nc.vector.tensor_tensor(hn[:], hn[:], b_t[:], OP.add)
                return hn

            fg = fb = None
            if use_gb:
                fg = gbp.tile([128, H], BF16, tag="g", name="gfin")
                nc.scalar.dma_start(fg[:], fing[None, :].to_broadcast((128, H)))
                fb = gbp.tile([128, H], BF16, tag="b", name="bfin")
                nc.scalar.dma_start(fb[:], finb[None, :].to_broadcast((128, H)))

            hTfin = [None] * TT

            def emit_final(t):
                """Final LN -> transposed head input for tile t.

                When the final affine is trivial (this input), LN of an
                LN output is the identity to ~1e-6 relative — skip it and
                transpose the layer-8 LN output directly.
                """
                h8 = h_cur[t]
                if not use_gb:
                    hTfin[t] = cast_transpose(h8, head_sigma, hTp, f"fin{t}", t)
                    return
                if t == 0:
                    fsums = smp.tile(
                        [128, 1 + NCH], F32, tag="fsums", name=f"smfin{t}"
                    )
                    nc.vector.tensor_reduce(
                        fsums[:, 0:1], h8[:], axis=AX.X, op=OP.add
                    )
                    for i in range(NCH):
                        dump = psY.tile(
                            [128, CH], F32, tag="psY", name=f"dmp{t}_{i}"
                        )
                        nc.scalar.activation(
                            dump[:],
                            h8[:, i * CH : (i + 1) * CH],
                            AF.Square,
                            accum_out=fsums[:, 1 + i : 2 + i],
                        )
                    hfin = ln_finish(
                        h8, fsums[:, 0:1], fsums[:, 1 : 1 + NCH],
                        fg, fb, f"fin{t}",
                    )
                else:
                    fsums = smp.tile([128, 2], F32, tag="fsums1", name=f"smfin{t}")
                    nc.vector.tensor_reduce(
                        fsums[:, 0:1], h8[:], axis=AX.X, op=OP.add
                    )
                    fsq = zpool.tile([128, H], F32, tag="z", name=f"fsq{t}")
                    nc.vector.tensor_tensor(fsq[:], h8[:], h8[:], OP.mult)
                    nc.vector.tensor_reduce(
                        fsums[:, 1:2], fsq[:], axis=AX.X, op=OP.add
                    )
                    hfin = ln_finish(
                        h8, fsums[:, 0:1], fsums[:, 1:2], fg, fb, f"fin{t}"
                    )
                hTfin[t] = cast_transpose(hfin, head_sigma, hTp, f"fin{t}", t)

            EARLY_PE = 4  # transposes for layers 1..4 run on the PE — the
            # DMA queues are bandwidth-starved while the weight prefetch
            # backlog drains
            pending = [None, None]
            gbt = {}
            hw_pre = []  # first head chunks, prefetched under the last layer
            for l in range(L):
                if l > 1:
                    w_half = load_w(l)
                elif l == 1:
                    w_half = w_half1
                else:
                    w_half = w_half0
                if l == L - 1:
                    for ch in range(2):
                        hwt = hwp.tile(
                            [128, KT, CV], FP8, tag="hw", name=f"hw{ch}"
                        )
                        nc.scalar.dma_start(
                            hwt[:],
                            hw_[:, :, ch * CV : (ch + 1) * CV].rearrange(
                                "k p v -> p k v"
                            ),
                        )
                        hw_pre.append(hwt)
                if use_gb:
                    g_t = gbp.tile([128, H], BF16, tag="g", name=f"g{l}")
                    nc.scalar.dma_start(g_t[:], lng[l][None, :].to_broadcast((128, H)))
                    b_t = gbp.tile([128, H], BF16, tag="b", name=f"b{l}")
                    nc.scalar.dma_start(b_t[:], lnb[l][None, :].to_broadcast((128, H)))
                    bias_t = gbp.tile([128, H], BF16, tag="bias", name=f"bias{l}")
                    nc.scalar.dma_start(
                        bias_t[:], lbias[l][None, :].to_broadcast((128, H))
                    )
                    gbt = dict(g=g_t, b=b_t, bias=bias_t)

                for t in range(TT):
                    hTt = hT[t]
                    ps = []
                    for i in range(NCH):
                        p = psY.tile([128, CH], F32, tag="psY", name=f"ps{l}_{t}_{i}")
                        ps.append(p)
                    for kt in range(KT):
                        if kt == KT // 2 and pending[1 - t] is not None:
                            pending[1 - t]()
                            pending[1 - t] = None
                        wt = w_half[kt // (KT // 4)]
                        for i in range(NCH):
                            nc.tensor.matmul(
                                ps[i][:],
                                lhsT=hTt[:, kt, :],
                                rhs=wt[:, kt % (KT // 4), i * CH : (i + 1) * CH],
                                start=(kt == 0),
                                stop=(kt == KT - 1),
                                skip_group_check=True,
                            )
                    z = zpool.tile([128, H], F32, tag="z", name=f"z{l}_{t}")
                    sums = smp.tile([128, 1 + NCH], F32, tag="sums", name=f"sm{l}_{t}")
                    resid = h_cur[t]
                    if use_gb:
                        hb = zpool.tile([128, H], F32, tag="hb", name=f"hb{l}_{t}")
                        nc.vector.tensor_tensor(hb[:], h_cur[t][:], gbt["bias"][:], OP.add)
                        resid = hb
                    for i in range(NCH):
                        nc.vector.tensor_add(
                            z[:, i * CH : (i + 1) * CH],
                            ps[i][:],
                            resid[:, i * CH : (i + 1) * CH],
                        )
                    nc.vector.tensor_reduce(sums[:, 0:1], z[:], axis=AX.X, op=OP.add)
                    for i in range(NCH):
                        nc.scalar.activation(
                            ps[i][:],
                            z[:, i * CH : (i + 1) * CH],
                            AF.Square,
                            accum_out=sums[:, 1 + i : 2 + i],
                        )
                    h_cur[t] = ln_finish(
                        z, sums[:, 0:1], sums[:, 1 : 1 + NCH],
                        gbt.get("g"), gbt.get("b"), f"{l}_{t}",
                    )
                    if l + 1 < L:
                        if l + 1 <= EARLY_PE:
                            def mk(tt, ll, src):
                                def emit():
                                    hT[tt] = pe_transpose(
                                        src, sigmas[ll + 1], hTp, f"{ll + 1}_{tt}"
                                    )
                                return emit
                            pending[t] = mk(t, l, h_cur[t])
                        else:
                            # emitted NOW: the cast sits right behind this
                            # tile's LN in the vector FIFO and the XBAR runs
                            # during the other tile's matmuls
                            hT[t] = cast_transpose(
                                h_cur[t], sigmas[l + 1], hTp, f"{l + 1}_{t}", t
                            )
                    else:
                        emit_final(t)

            # head: stream the full lm_head in CV-wide vocab chunks; each
            # chunk serves both token tiles (compute:DMA ~ 2.3:1)
            for ch in range(NCHK):
                if ch < 2:
                    hwt = hw_pre[ch]
                else:
                    # head weights stream on the otherwise-idle sync queue so
                    # their issue never queues behind the PSUM->staging
                    # copies (which block awaiting matmul completion)
                    hwt = hwp.tile([128, KT, CV], FP8, tag="hw", name=f"hw{ch}")
                    nc.sync.dma_start(
                        hwt[:],
                        hw_[:, :, ch * CV : (ch + 1) * CV].rearrange("k p v -> p k v"),
                    )
                for t in range(TT):
                    pss = [
                        psY.tile([128, CH], F32, tag="psY", name=f"ph{ch}_{t}_{v}")
                        for v in range(NVC)
                    ]
                    for kt in range(KT):
                        for vi in range(NVC):
                            nc.tensor.matmul(
                                pss[vi][:, 0:NV],
                                lhsT=hTfin[t][:, kt, :],
                                rhs=hwt[:, kt, vi * NV : (vi + 1) * NV],
                                start=(kt == 0),
                                stop=(kt == KT - 1),
                                skip_group_check=True,
                            )
                    o_t = outp.tile([128, CV], BF16, tag="ostg", name=f"o{ch}_{t}")
                    last = ch == NCHK - 1 and t == TT - 1
                    for vi in range(NVC):
                        dst = o_t[:, vi * NV : (vi + 1) * NV]
                        if last and vi % 2 == 1:
                            # final unit: split copies across both engines and
                            # drain in two DMAs so the kernel tail is shorter
                            nc.vector.tensor_scalar_mul(dst, pss[vi][:, 0:NV], 1.0)
                        else:
                            nc.scalar.copy(dst, pss[vi][:, 0:NV])
                        if last and vi == 1:
                            nc.scalar.dma_start(
                                out[t * 128 : (t + 1) * 128,
                                    ch * CV : ch * CV + 2 * NV],
                                o_t[:, 0 : 2 * NV],
                            )
                    if last:
                        nc.scalar.dma_start(
                            out[t * 128 : (t + 1) * 128,
                                ch * CV + 2 * NV : (ch + 1) * CV],
                            o_t[:, 2 * NV : CV],
                        )
                    else:
                        # out rides scalar, directly behind its own copies
                        nc.scalar.dma_start(
                            out[t * 128 : (t + 1) * 128, ch * CV : (ch + 1) * CV],
                            o_t[:],
                        )

    return nc


def _ternary(wmat):
    """Exact {-1,0,1} ternary tensor + fp32 scale, matching the reference."""
    w = np.asarray(wmat, dtype=np.float32)
    s = np.mean(np.abs(w), dtype=np.float32)
    t = np.clip(np.rint(w / (s + np.float32(1e-8))), -1.0, 1.0).astype(np.float32)
    return t, float(s)


def _split_scale(s):
    """s = sigma * 2^-e with sigma ~ 1 and 2^-e exact in fp8e4."""
    e = int(np.clip(np.round(-np.log2(s)), -7, 9))
    return s * (2.0**e), e


_NC_CACHE = {}
_LAST_RESULTS = None


def kernel(**inputs):
    global _LAST_RESULTS
    cfg = CFG_FULL
    L, H, NTOK, NC, TT, V = (
        cfg["L"], cfg["H"], cfg["NTOK"], cfg["NC"], cfg["TT"], cfg["V"],
    )
    KT = H // 128
    TPC = TT * 128  # tokens per core
    BF = ml_dtypes.bfloat16
    F8 = ml_dtypes.float8_e4m3fn

    ids = np.asarray(inputs["input_ids"]).astype(np.int64).reshape(-1)
    embed = np.asarray(inputs["embed"], dtype=np.float32)
    layer_w = np.asarray(inputs["layer_w"], dtype=np.float32)
    layer_b = np.asarray(inputs["layer_b"], dtype=np.float32)
    ln_g = np.asarray(inputs["ln_g"], dtype=np.float32)
    ln_b = np.asarray(inputs["ln_b"], dtype=np.float32)
    final_g = np.asarray(inputs["final_g"], dtype=np.float32)
    final_b = np.asarray(inputs["final_b"], dtype=np.float32)
    head_w = np.asarray(inputs["head_w"], dtype=np.float32)

    use_gb = not (
        np.all(layer_b == 0.0)
        and np.all(ln_g == 1.0)
        and np.all(ln_b == 0.0)
        and np.all(final_g == 1.0)
        and np.all(final_b == 0.0)
    )

    h0_full = embed[ids]  # [NTOK, H] fp32

    sigmas = []
    wT = np.empty([L, KT, 128, H], dtype=F8)
    for l in range(L):
        t, s = _ternary(layer_w[l])
        sig, e = _split_scale(s)
        sigmas.append(sig)
        wT[l] = (
            (np.ascontiguousarray(t.T) * np.float32(2.0**-e))
            .reshape(KT, 128, H)
            .astype(F8)
        )
    th, head_scale = _ternary(head_w)
    head_sigma, e_h = _split_scale(head_scale)
    headT = (
        (np.ascontiguousarray(th.T) * np.float32(2.0**-e_h))
        .reshape(KT, 128, -1)
        .astype(F8)
    )  # [KT, 128, V]

    key = (id(cfg), tuple(sigmas), head_sigma, use_gb)
    if key not in _NC_CACHE:
        _NC_CACHE.clear()
        nc = build_nc(cfg, sigmas, head_sigma, use_gb)
        # Bacc.finalize runs the TRN2 legalization passes (1-wait-per-
        # instruction event-semaphore split, matmul->ldweights wait motion,
        # register allocation). The PJRT exec path serializes nc as-is.
        nc.finalize()
        _NC_CACHE[key] = nc
    nc = _NC_CACHE[key]

    common = {
        "w": wT,
        "hw": headT,
        "ident": np.eye(128, dtype=np.float32),
        "eps": np.full((128, 1), EPS, np.float32),
    }
    if use_gb:
        common.update(
            lng=ln_g.astype(BF),
            lnb=ln_b.astype(BF),
            lbias=layer_b.astype(BF),
            fing=final_g.astype(BF),
            finb=final_b.astype(BF),
        )
    in_maps = []
    for c in range(NC):
        h0c = np.ascontiguousarray(
            h0_full[c * TPC : (c + 1) * TPC].reshape(TT, 128, H)
        )
        # layer-0 transposed+scaled input, in XBAR block layout
        h0t = np.ascontiguousarray(
            (h0c * np.float32(sigmas[0]))
            .transpose(0, 2, 1)  # [TT, H, 128tok]
            .reshape(TT, KT, 128, 128)
            .transpose(0, 2, 1, 3)  # [TT, 128p, KT, 128tok]
        ).astype(np.float16)
        in_maps.append(dict(common, h0=h0c.astype(BF), h0T=h0t))

    trace = bool(int(os.environ.get("TRIKERNEL_TRACE", "0")))
    res = run_bass_kernel_spmd(nc, in_maps, core_ids=list(range(NC)), trace=trace)
    _LAST_RESULTS = res

    full = np.concatenate(
        [np.asarray(res.results[c]["out"]) for c in range(NC)], axis=0
    )  # [NTOK, V] bf16
    return full.reshape(2, 1024, 32000).astype(np.float32)


# revision 59
# speedup vs baseline: 1.0629x; 1.0210x over previous
"""Trainium2 Bass kernel: 8-layer ternary (BitNet-1.58) dense transformer.

Model (per reference):
    h = embed[input_ids]                                  # (B=2, S=1024, H=2048)
    8x: y = h @ ternary(W_l)^T + b_l ; h = LN(y + h)*g+b  # H=2048
    h = LN(h)*final_g + final_b
    logits = h @ ternary(head_W)^T                        # (B, S, V=32000)

Sharding over 8 NeuronCores: fully data-parallel over the 2048 tokens
(256 tokens/core). Each core streams the full layer weights (fp8, 33 MB)
during the layer phase and the full lm_head (fp8, 65 MB) during the head
phase; the head stays compute-bound (2.3x margin over DMA at 358 GB/s).
There are NO collectives — cores never exchange data, which removes the
AllGather straggler coupling (per-core clock-throttle variance made a
tensor-parallel head's gather completion highly variable) and the
collective bootstrap barrier (whose ring ownership blocks XBAR
transposes).

Precision: ternary weights are scaled by an exact power of two (2^-e, e~6)
so they are EXACT in fp8e4; the compensating factor (s * 2^e ~= 1) is folded
into the activation cast. Matmuls run fp16 (activations, stationary) x
fp8 (weights, moving); PSUM accumulates fp32; the residual/LN path stays
fp32. Logits are written bf16 and upcast on the host.

Scheduling:
  - The PE runs matmuls ONLY. Activation transposes go through the DMA XBAR
    (dma_start(transpose=True), fp16, SBUF->SBUF, out[p, kt, tok] =
    in[tok, kt*128 + p] — verified against CoreSim's InstDmaTransposeAnt),
    alternating between the two HWDGE queues (sync/scalar) by tile parity.
  - Software pipelining by emission order: each tile's next-layer cast is
    emitted directly after its ln_finish, so the (strict-FIFO) vector engine
    runs it before the other tile's LN chain, and the XBAR transpose lands
    during the other tile's matmuls. Layer-0 transposed inputs come from
    the host ("h0T"), so the first matmul waits only on two small DMAs.
  - Weights ride the scalar-engine DGE queue; transposes and small loads
    ride the sync queue.
  - The board power governor caps sustained near-100%-duty matmul streams
    at K=13/16 (~1.95 GHz, type-31 HAM events) with large run-to-run
    variance; minimizing total PE cycles is what matters, so transposes are
    kept off the PE even though the PE has idle slots for them.
"""

import os
import sys

import numpy as np

try:
    import concourse.bass as bass
except ImportError:  # grading container should have it on sys.path already
    sys.path.insert(0, "/opt/trn_rl_repo")
    import concourse.bass as bass

import ml_dtypes
import concourse.mybir as mybir
import concourse.tile as tile
from concourse import bacc
from concourse.bass_utils import run_bass_kernel_spmd
from contextlib import ExitStack

F32 = mybir.dt.float32
BF16 = mybir.dt.bfloat16
FP16 = mybir.dt.float16
FP8 = mybir.dt.float8e4
AX = mybir.AxisListType
OP = mybir.AluOpType
AF = mybir.ActivationFunctionType
EPS = 1e-5

# Full-size problem config (B=2, S=1024 -> 2048 tokens).
# CV: vocab chunk streamed per head step.
CFG_FULL = dict(L=8, H=2048, NTOK=2048, NC=8, TT=2, V=32000, NV=500, CH=512,
                CV=2000)


def build_nc(cfg, sigmas, head_sigma, use_gb):
    L, H, NTOK, NC, TT = cfg["L"], cfg["H"], cfg["NTOK"], cfg["NC"], cfg["TT"]
    V, NV, CH, CV = cfg["V"], cfg["NV"], cfg["CH"], cfg["CV"]
    KT = H // 128
    KH = KT // 2  # k-tiles per weight half
    NCH = H // CH
    NVC = CV // NV  # vocab sub-chunks per streamed chunk (4)
    NCHK = V // CV  # streamed head chunks (16)
    TPC = TT * 128
    assert NTOK == NC * TPC

    nc = bacc.Bacc("TRN2", target_bir_lowering=False, debug=False, num_devices=NC)
    h0 = nc.declare_dram_parameter("h0", [TT, 128, H], BF16, isOutput=False)
    h0T = nc.declare_dram_parameter("h0T", [TT, 128, KT, 128], FP16, isOutput=False)
    w_ = nc.declare_dram_parameter("w", [L, KT, 128, H], FP8, isOutput=False)
    if use_gb:
        lng = nc.declare_dram_parameter("lng", [L, H], BF16, isOutput=False)
        lnb = nc.declare_dram_parameter("lnb", [L, H], BF16, isOutput=False)
        lbias = nc.declare_dram_parameter("lbias", [L, H], BF16, isOutput=False)
        fing = nc.declare_dram_parameter("fing", [H], BF16, isOutput=False)
        finb = nc.declare_dram_parameter("finb", [H], BF16, isOutput=False)
    hw_ = nc.declare_dram_parameter("hw", [KT, 128, V], FP8, isOutput=False)
    ident_d = nc.declare_dram_parameter("ident", [128, 128], F32, isOutput=False)
    eps_d = nc.declare_dram_parameter("eps", [128, 1], F32, isOutput=False)
    out = nc.declare_dram_parameter("out", [TPC, V], BF16, isOutput=True)

    with tile.TileContext(nc) as tc:
        with ExitStack() as ctxA:
            consts = ctxA.enter_context(tc.tile_pool(name="consts", bufs=1))
            state = ctxA.enter_context(tc.tile_pool(name="state", bufs=3))
            zpool = ctxA.enter_context(tc.tile_pool(name="z", bufs=2))
            hscp = ctxA.enter_context(tc.tile_pool(name="hsc", bufs=2))
            hTp = ctxA.enter_context(tc.tile_pool(name="hT", bufs=2))
            wp = ctxA.enter_context(tc.tile_pool(name="w", bufs=8))
            hwp = ctxA.enter_context(tc.tile_pool(name="hw", bufs=2))
            outp = ctxA.enter_context(tc.tile_pool(name="outstg", bufs=4))
            gbp = None
            if use_gb:
                gbp = ctxA.enter_context(tc.tile_pool(name="gb", bufs=2))
            smp = ctxA.enter_context(tc.tile_pool(name="small", bufs=16))
            psY = ctxA.enter_context(
                tc.tile_pool(name="psY", bufs=2 * NCH, space="PSUM")
            )

            eps_t = consts.tile([128, 1], F32)
            nc.sync.dma_start(eps_t[:], eps_d[:])
            ident = consts.tile([128, 128], F32)
            nc.sync.dma_start(ident[:], ident_d[:])

            def load_w(l):
                """Layer weights in 4-ktile quarters — fine granularity lets
                each layer's first matmuls start before the whole 4.2 MB
                layer arrives (the early phase is DMA-bandwidth-starved).
                All on the scalar queue: mixing weight DMAs onto the sync
                queue alongside XBAR transposes corrupts transfers.
                """
                qs = []
                for qf in range(4):
                    wt = wp.tile([128, KT // 4, H], FP8, tag="w", name=f"w{l}_{qf}")
                    # layers 0-3: split quarters across BOTH queues — the
                    # early phase is bandwidth-starved and the sync queue
                    # carries no XBARs until ~t=115us (EARLY_PE covers the
                    # overlap window; concurrent XBAR+bulk on sync corrupts)
                    eng = nc.sync if (l <= 3 and qf % 2 == 1) else nc.scalar
                    eng.dma_start(
                        wt[:],
                        w_[l, qf * (KT // 4) : (qf + 1) * (KT // 4)].rearrange(
                            "k p o -> p k o"
                        ),
                    )
                    qs.append(wt)
                return qs

            w_half0 = load_w(0)

            h_cur = []
            hT = []
            for t in range(TT):
                # layer-0 transposed input comes precomputed from the host
                ht = hTp.tile([128, KT, 128], FP16, tag="hT", name=f"hTp{t}")
                nc.sync.dma_start(ht[:], h0T[t])
                hT.append(ht)
                st = state.tile([128, H], BF16, name=f"hinit{t}", tag="state")
                h_cur.append(st)
            # residual loads: plenty of slack (first needed ~35us in)
            for t in range(TT):
                nc.scalar.dma_start(h_cur[t][:], h0[t])
            w_half1 = load_w(1)

            def cast_transpose(src_f32, scale_imm, pool, name, t):
                """h [128tok, H] f32 -> hT [128, KT, 128tok] fp16 * scale."""
                hsc = hscp.tile([128, H], FP16, tag="hsc", name=f"hsc{name}")
                nc.vector.tensor_scalar_mul(hsc[:], src_f32[:], float(scale_imm))
                dst = pool.tile([128, KT, 128], FP16, tag="hT", name=f"hT{name}")
                # layer XBARs all ride sync (no weight traffic there, so no
                # ring-corruption hazard, and they stop delaying the weight
                # quarter issues queued on scalar). Tile 1's FINAL transpose
                # stays on scalar: it completes early in the head phase and
                # must not overlap the head-weight DMAs starting on sync.
                eng = nc.scalar if name == "fin1" else nc.sync
                eng.dma_start(dst[:], hsc[:], transpose=True)
                return dst

            def pe_transpose(src_f32, scale_imm, pool, name):
                """PE-based transpose via psY chunks — used for the early
                layers, when the DMA queues are bandwidth-starved."""
                dst = pool.tile([128, KT, 128], FP16, tag="hT", name=f"hT{name}")
                for j in range(NCH):
                    pc = psY.tile([128, CH], F32, tag="psY", name=f"pT{name}_{j}")
                    for u in range(CH // 128):
                        kt = j * (CH // 128) + u
                        nc.tensor.transpose(
                            pc[:, u * 128 : (u + 1) * 128],
                            src_f32[:, kt * 128 : (kt + 1) * 128],
                            ident[:],
                        )
                    nc.scalar.activation(
                        dst[:, j * (CH // 128) : (j + 1) * (CH // 128), :],
                        pc[:],
                        AF.Copy,
                        scale=float(scale_imm),
                    )
                return dst

            def ln_finish(affine_src, S_ap, SS_ap, g_t, b_t, name):
                S = smp.tile([128, 1], F32, tag="s0", name=f"S{name}")
                SS = smp.tile([128, 1], F32, tag="s1", name=f"SS{name}")
                nc.vector.tensor_reduce(S[:], S_ap, axis=AX.X, op=OP.add)
                nc.vector.tensor_reduce(SS[:], SS_ap, axis=AX.X, op=OP.add)
                negmean = smp.tile([128, 1], F32, tag="s2", name=f"nm{name}")
                nc.vector.tensor_scalar_mul(negmean[:], S[:], -1.0 / H)
                msq = smp.tile([128, 1], F32, tag="s3", name=f"msq{name}")
                nc.vector.tensor_scalar_mul(msq[:], SS[:], 1.0 / H)
                var = smp.tile([128, 1], F32, tag="s4", name=f"var{name}")
                nc.vector.tensor_tensor(var[:], negmean[:], negmean[:], OP.mult)
                nc.vector.tensor_tensor(var[:], msq[:], var[:], OP.subtract)
                std = smp.tile([128, 1], F32, tag="s5", name=f"std{name}")
                nc.scalar.activation(std[:], var[:], AF.Sqrt, bias=eps_t[:])
                rstd = smp.tile([128, 1], F32, tag="s6", name=f"rstd{name}")
                nc.vector.reciprocal(rstd[:], std[:])
                hn = state.tile([128, H], F32, tag="state", name=f"h{name}")
                nc.vector.tensor_scalar(
                    hn[:], affine_src[:], negmean[:], rstd[:], OP.add, OP.mult
                )
                if g_t is not None:
                    nc.vector.tensor_tensor(hn[:], hn[:], g_t[:], OP.mult)
                    nc.vector.tensor_tensor(hn[:], hn[:], b_t[:], OP.add)
                return hn

            fg = fb = None
            if use_gb:
                fg = gbp.tile([128, H], BF16, tag="g", name="gfin")
                nc.scalar.dma_start(fg[:], fing[None, :].to_broadcast((128, H)))
                fb = gbp.tile([128, H], BF16, tag="b", name="bfin")
                nc.scalar.dma_start(fb[:], finb[None, :].to_broadcast((128, H)))

            hTfin = [None] * TT

            def emit_final(t):
                """Final LN -> transposed head input for tile t.

                When the final affine is trivial (this input), LN of an
                LN output is the identity to ~1e-6 relative — skip it and
                transpose the layer-8 LN output directly.
                """
                h8 = h_cur[t]
                if not use_gb:
                    hTfin[t] = cast_transpose(h8, head_sigma, hTp, f"fin{t}", t)
                    return
                if t == 0:
                    fsums = smp.tile(
                        [128, 1 + NCH], F32, tag="fsums", name=f"smfin{t}"
                    )
                    nc.vector.tensor_reduce(
                        fsums[:, 0:1], h8[:], axis=AX.X, op=OP.add
                    )
                    for i in range(NCH):
                        dump = psY.tile(
                            [128, CH], F32, tag="psY", name=f"dmp{t}_{i}"
                        )
                        nc.scalar.activation(
                            dump[:],
                            h8[:, i * CH : (i + 1) * CH],
                            AF.Square,
                            accum_out=fsums[:, 1 + i : 2 + i],
                        )
                    hfin = ln_finish(
                        h8, fsums[:, 0:1], fsums[:, 1 : 1 + NCH],
                        fg, fb, f"fin{t}",
                    )
                else:
                    fsums = smp.tile([128, 2], F32, tag="fsums1", name=f"smfin{t}")
                    nc.vector.tensor_reduce(
                        fsums[:, 0:1], h8[:], axis=AX.X, op=OP.add
                    )
                    fsq = zpool.tile([128, H], F32, tag="z", name=f"fsq{t}")
                    nc.vector.tensor_tensor(fsq[:], h8[:], h8[:], OP.mult)
                    nc.vector.tensor_reduce(
                        fsums[:, 1:2], fsq[:], axis=AX.X, op=OP.add
                    )
                    hfin = ln_finish(
                        h8, fsums[:, 0:1], fsums[:, 1:2], fg, fb, f"fin{t}"
                    )
                hTfin[t] = cast_transpose(hfin, head_sigma, hTp, f"fin{t}", t)

            EARLY_PE = 4  # transposes for layers 1..4 run on the PE — the
            # DMA queues are bandwidth-starved while the weight prefetch
            # backlog drains
            pending = [None, None]
            gbt = {}
            hw_pre = []  # first head chunks, prefetched under the last layer
            for l in range(L):
                if l > 1:
                    w_half = load_w(l)
                elif l == 1:
                    w_half = w_half1
                else:
                    w_half = w_half0
                if l == L - 1:
                    for ch in range(2):
                        hwt = hwp.tile(
                            [128, KT, CV], FP8, tag="hw", name=f"hw{ch}"
                        )
                        nc.scalar.dma_start(
                            hwt[:],
                            hw_[:, :, ch * CV : (ch + 1) * CV].rearrange(
                                "k p v -> p k v"
                            ),
                        )
                        hw_pre.append(hwt)
                if use_gb:
                    g_t = gbp.tile([128, H], BF16, tag="g", name=f"g{l}")
                    nc.scalar.dma_start(g_t[:], lng[l][None, :].to_broadcast((128, H)))
                    b_t = gbp.tile([128, H], BF16, tag="b", name=f"b{l}")
                    nc.scalar.dma_start(b_t[:], lnb[l][None, :].to_broadcast((128, H)))
                    bias_t = gbp.tile([128, H], BF16, tag="bias", name=f"bias{l}")
                    nc.scalar.dma_start(
                        bias_t[:], lbias[l][None, :].to_broadcast((128, H))
                    )
                    gbt = dict(g=g_t, b=b_t, bias=bias_t)

                for t in range(TT):
                    hTt = hT[t]
                    ps = []
                    for i in range(NCH):
                        p = psY.tile([128, CH], F32, tag="psY", name=f"ps{l}_{t}_{i}")
                        ps.append(p)
                    for kt in range(KT):
                        if kt == KT // 2 and pending[1 - t] is not None:
                            pending[1 - t]()
                            pending[1 - t] = None
                        wt = w_half[kt // (KT // 4)]
                        for i in range(NCH):
                            nc.tensor.matmul(
                                ps[i][:],
                                lhsT=hTt[:, kt, :],
                                rhs=wt[:, kt % (KT // 4), i * CH : (i + 1) * CH],
                                start=(kt == 0),
                                stop=(kt == KT - 1),
                                skip_group_check=True,
                            )
                    z = zpool.tile([128, H], F32, tag="z", name=f"z{l}_{t}")
                    sums = smp.tile([128, 1 + NCH], F32, tag="sums", name=f"sm{l}_{t}")
                    resid = h_cur[t]
                    if use_gb:
                        hb = zpool.tile([128, H], F32, tag="hb", name=f"hb{l}_{t}")
                        nc.vector.tensor_tensor(hb[:], h_cur[t][:], gbt["bias"][:], OP.add)
                        resid = hb
                    for i in range(NCH):
                        nc.vector.tensor_add(
                            z[:, i * CH : (i + 1) * CH],
                            ps[i][:],
                            resid[:, i * CH : (i + 1) * CH],
                        )
                    nc.vector.tensor_reduce(sums[:, 0:1], z[:], axis=AX.X, op=OP.add)
                    for i in range(NCH):
                        nc.scalar.activation(
                            ps[i][:],
                            z[:, i * CH : (i + 1) * CH],
                            AF.Square,
                            accum_out=sums[:, 1 + i : 2 + i],
                        )
                    h_cur[t] = ln_finish(
                        z, sums[:, 0:1], sums[:, 1 : 1 + NCH],
                        gbt.get("g"), gbt.get("b"), f"{l}_{t}",
                    )
                    if l + 1 < L:
                        if l + 1 <= EARLY_PE:
                            def mk(tt, ll, src):
                                def emit():
                                    hT[tt] = pe_transpose(
                                        src, sigmas[ll + 1], hTp, f"{ll + 1}_{tt}"
                                    )
                                return emit
                            pending[t] = mk(t, l, h_cur[t])
                        else:
                            # emitted NOW: the cast sits right behind this
                            # tile's LN in the vector FIFO and the XBAR runs
                            # during the other tile's matmuls
                            hT[t] = cast_transpose(
                                h_cur[t], sigmas[l + 1], hTp, f"{l + 1}_{t}", t
                            )
                    else:
                        emit_final(t)

            # head: stream the full lm_head in CV-wide vocab chunks; each
            # chunk serves both token tiles (compute:DMA ~ 2.3:1)
            for ch in range(NCHK):
                if ch < 2:
                    hwt = hw_pre[ch]
                else:
                    # head weights stream on the otherwise-idle sync queue so
                    # their issue never queues behind the PSUM->staging
                    # copies (which block awaiting matmul completion)
                    hwt = hwp.tile([128, KT, CV], FP8, tag="hw", name=f"hw{ch}")
                    nc.sync.dma_start(
                        hwt[:],
                        hw_[:, :, ch * CV : (ch + 1) * CV].rearrange("k p v -> p k v"),
                    )
                for t in range(TT):
                    pss = [
                        psY.tile([128, CH], F32, tag="psY", name=f"ph{ch}_{t}_{v}")
                        for v in range(NVC)
                    ]
                    for kt in range(KT):
                        for vi in range(NVC):
                            nc.tensor.matmul(
                                pss[vi][:, 0:NV],
                                lhsT=hTfin[t][:, kt, :],
                                rhs=hwt[:, kt, vi * NV : (vi + 1) * NV],
                                start=(kt == 0),
                                stop=(kt == KT - 1),
                                skip_group_check=True,
                            )
                    o_t = outp.tile([128, CV], BF16, tag="ostg", name=f"o{ch}_{t}")
                    last = ch == NCHK - 1 and t == TT - 1
                    for vi in range(NVC):
                        dst = o_t[:, vi * NV : (vi + 1) * NV]
                        if last and vi % 2 == 1:
                            # final unit: split copies across both engines and
                            # drain in two DMAs so the kernel tail is shorter
                            nc.vector.tensor_scalar_mul(dst, pss[vi][:, 0:NV], 1.0)
                        else:
                            nc.scalar.copy(dst, pss[vi][:, 0:NV])
                        if last and vi == 1:
                            nc.scalar.dma_start(
                                out[t * 128 : (t + 1) * 128,
                                    ch * CV : ch * CV + 2 * NV],
                                o_t[:, 0 : 2 * NV],
                            )
                    if last:
                        nc.scalar.dma_start(
                            out[t * 128 : (t + 1) * 128,
                                ch * CV + 2 * NV : (ch + 1) * CV],
                            o_t[:, 2 * NV : CV],
                        )
                    else:
                        # out rides scalar, directly behind its own copies
                        nc.scalar.dma_start(
                            out[t * 128 : (t + 1) * 128, ch * CV : (ch + 1) * CV],
                            o_t[:],
                        )

    return nc


def _ternary(wmat):
    """Exact {-1,0,1} ternary tensor + fp32 scale, matching the reference."""
    w = np.asarray(wmat, dtype=np.float32)
    s = np.mean(np.abs(w), dtype=np.float32)
    t = np.clip(np.rint(w / (s + np.float32(1e-8))), -1.0, 1.0).astype(np.float32)
    return t, float(s)


def _split_scale(s):
    """s = sigma * 2^-e with sigma ~ 1 and 2^-e exact in fp8e4."""
    e = int(np.clip(np.round(-np.log2(s)), -7, 9))
    return s * (2.0**e), e


_NC_CACHE = {}
_LAST_RESULTS = None


def kernel(**inputs):
    global _LAST_RESULTS
    cfg = CFG_FULL
    L, H, NTOK, NC, TT, V = (
        cfg["L"], cfg["H"], cfg["NTOK"], cfg["NC"], cfg["TT"], cfg["V"],
    )
    KT = H // 128
    TPC = TT * 128  # tokens per core
    BF = ml_dtypes.bfloat16
    F8 = ml_dtypes.float8_e4m3fn

    ids = np.asarray(inputs["input_ids"]).astype(np.int64).reshape(-1)
    embed = np.asarray(inputs["embed"], dtype=np.float32)
    layer_w = np.asarray(inputs["layer_w"], dtype=np.float32)
    layer_b = np.asarray(inputs["layer_b"], dtype=np.float32)
    ln_g = np.asarray(inputs["ln_g"], dtype=np.float32)
    ln_b = np.asarray(inputs["ln_b"], dtype=np.float32)
    final_g = np.asarray(inputs["final_g"], dtype=np.float32)
    final_b = np.asarray(inputs["final_b"], dtype=np.float32)
    head_w = np.asarray(inputs["head_w"], dtype=np.float32)

    use_gb = not (
        np.all(layer_b == 0.0)
        and np.all(ln_g == 1.0)
        and np.all(ln_b == 0.0)
        and np.all(final_g == 1.0)
        and np.all(final_b == 0.0)
    )

    h0_full = embed[ids]  # [NTOK, H] fp32

    sigmas = []
    wT = np.empty([L, KT, 128, H], dtype=F8)
    for l in range(L):
        t, s = _ternary(layer_w[l])
        sig, e = _split_scale(s)
        sigmas.append(sig)
        wT[l] = (
            (np.ascontiguousarray(t.T) * np.float32(2.0**-e))
            .reshape(KT, 128, H)
            .astype(F8)
        )
    th, head_scale = _ternary(head_w)
    head_sigma, e_h = _split_scale(head_scale)
    headT = (
        (np.ascontiguousarray(th.T) * np.float32(2.0**-e_h))
        .reshape(KT, 128, -1)
        .astype(F8)
    )  # [KT, 128, V]

    key = (id(cfg), tuple(sigmas), head_sigma, use_gb)
    if key not in _NC_CACHE:
        _NC_CACHE.clear()
        nc = build_nc(cfg, sigmas, head_sigma, use_gb)
        # Bacc.finalize runs the TRN2 legalization passes (1-wait-per-
        # instruction event-semaphore split, matmul->ldweights wait motion,
        # register allocation). The PJRT exec path serializes nc as-is.
        nc.finalize()
        _NC_CACHE[key] = nc
    nc = _NC_CACHE[key]

    common = {
        "w": wT,
        "hw": headT,
        "ident": np.eye(128, dtype=np.float32),
        "eps": np.full((128, 1), EPS, np.float32),
    }
    if use_gb:
        common.update(
            lng=ln_g.astype(BF),
            lnb=ln_b.astype(BF),
            lbias=layer_b.astype(BF),
            fing=final_g.astype(BF),
            finb=final_b.astype(BF),
        )
    in_maps = []
    for c in range(NC):
        h0c = np.ascontiguousarray(
            h0_full[c * TPC : (c + 1) * TPC].reshape(TT, 128, H)
        )
        # layer-0 transposed+scaled input, in XBAR block layout
        h0t = np.ascontiguousarray(
            (h0c * np.float32(sigmas[0]))
            .transpose(0, 2, 1)  # [TT, H, 128tok]
            .reshape(TT, KT, 128, 128)
            .transpose(0, 2, 1, 3)  # [TT, 128p, KT, 128tok]
        ).astype(np.float16)
        in_maps.append(dict(common, h0=h0c.astype(BF), h0T=h0t))

    trace = bool(int(os.environ.get("TRIKERNEL_TRACE", "0")))
    res = run_bass_kernel_spmd(nc, in_maps, core_ids=list(range(NC)), trace=trace)
    _LAST_RESULTS = res

    full = np.concatenate(
        [np.asarray(res.results[c]["out"]) for c in range(NC)], axis=0
    )  # [NTOK, V] bf16
    return full.reshape(2, 1024, 32000).astype(np.float32)


# revision 60
# speedup vs baseline: 1.0644x; 1.0014x over previous
"""Trainium2 Bass kernel: 8-layer ternary (BitNet-1.58) dense transformer.

Model (per reference):
    h = embed[input_ids]                                  # (B=2, S=1024, H=2048)
    8x: y = h @ ternary(W_l)^T + b_l ; h = LN(y + h)*g+b  # H=2048
    h = LN(h)*final_g + final_b
    logits = h @ ternary(head_W)^T                        # (B, S, V=32000)

Sharding over 8 NeuronCores: fully data-parallel over the 2048 tokens
(256 tokens/core). Each core streams the full layer weights (fp8, 33 MB)
during the layer phase and the full lm_head (fp8, 65 MB) during the head
phase; the head stays compute-bound (2.3x margin over DMA at 358 GB/s).
There are NO collectives — cores never exchange data, which removes the
AllGather straggler coupling (per-core clock-throttle variance made a
tensor-parallel head's gather completion highly variable) and the
collective bootstrap barrier (whose ring ownership blocks XBAR
transposes).

Precision: ternary weights are scaled by an exact power of two (2^-e, e~6)
so they are EXACT in fp8e4; the compensating factor (s * 2^e ~= 1) is folded
into the activation cast. Matmuls run fp16 (activations, stationary) x
fp8 (weights, moving); PSUM accumulates fp32; the residual/LN path stays
fp32. Logits are written bf16 and upcast on the host.

Scheduling:
  - The PE runs matmuls ONLY. Activation transposes go through the DMA XBAR
    (dma_start(transpose=True), fp16, SBUF->SBUF, out[p, kt, tok] =
    in[tok, kt*128 + p] — verified against CoreSim's InstDmaTransposeAnt),
    alternating between the two HWDGE queues (sync/scalar) by tile parity.
  - Software pipelining by emission order: each tile's next-layer cast is
    emitted directly after its ln_finish, so the (strict-FIFO) vector engine
    runs it before the other tile's LN chain, and the XBAR transpose lands
    during the other tile's matmuls. Layer-0 transposed inputs come from
    the host ("h0T"), so the first matmul waits only on two small DMAs.
  - Weights ride the scalar-engine DGE queue; transposes and small loads
    ride the sync queue.
  - The board power governor caps sustained near-100%-duty matmul streams
    at K=13/16 (~1.95 GHz, type-31 HAM events) with large run-to-run
    variance; minimizing total PE cycles is what matters, so transposes are
    kept off the PE even though the PE has idle slots for them.
"""

import os
import sys

import numpy as np

try:
    import concourse.bass as bass
except ImportError:  # grading container should have it on sys.path already
    sys.path.insert(0, "/opt/trn_rl_repo")
    import concourse.bass as bass

import ml_dtypes
import concourse.mybir as mybir
import concourse.tile as tile
from concourse import bacc
from concourse.bass_utils import run_bass_kernel_spmd
from contextlib import ExitStack

F32 = mybir.dt.float32
BF16 = mybir.dt.bfloat16
FP16 = mybir.dt.float16
FP8 = mybir.dt.float8e4
AX = mybir.AxisListType
OP = mybir.AluOpType
AF = mybir.ActivationFunctionType
EPS = 1e-5

# Full-size problem config (B=2, S=1024 -> 2048 tokens).
# CV: vocab chunk streamed per head step.
CFG_FULL = dict(L=8, H=2048, NTOK=2048, NC=8, TT=2, V=32000, NV=500, CH=512,
                CV=2000)


def build_nc(cfg, sigmas, head_sigma, use_gb):
    L, H, NTOK, NC, TT = cfg["L"], cfg["H"], cfg["NTOK"], cfg["NC"], cfg["TT"]
    V, NV, CH, CV = cfg["V"], cfg["NV"], cfg["CH"], cfg["CV"]
    KT = H // 128
    KH = KT // 2  # k-tiles per weight half
    NCH = H // CH
    NVC = CV // NV  # vocab sub-chunks per streamed chunk (4)
    NCHK = V // CV  # streamed head chunks (16)
    TPC = TT * 128
    assert NTOK == NC * TPC

    nc = bacc.Bacc("TRN2", target_bir_lowering=False, debug=False, num_devices=NC)
    h0 = nc.declare_dram_parameter("h0", [TT, 128, H], BF16, isOutput=False)
    h0T = nc.declare_dram_parameter("h0T", [TT, 128, KT, 128], FP16, isOutput=False)
    w_ = nc.declare_dram_parameter("w", [L, KT, 128, H], FP8, isOutput=False)
    if use_gb:
        lng = nc.declare_dram_parameter("lng", [L, H], BF16, isOutput=False)
        lnb = nc.declare_dram_parameter("lnb", [L, H], BF16, isOutput=False)
        lbias = nc.declare_dram_parameter("lbias", [L, H], BF16, isOutput=False)
        fing = nc.declare_dram_parameter("fing", [H], BF16, isOutput=False)
        finb = nc.declare_dram_parameter("finb", [H], BF16, isOutput=False)
    hw_ = nc.declare_dram_parameter("hw", [KT, 128, V], FP8, isOutput=False)
    ident_d = nc.declare_dram_parameter("ident", [128, 128], F32, isOutput=False)
    eps_d = nc.declare_dram_parameter("eps", [128, 1], F32, isOutput=False)
    out = nc.declare_dram_parameter("out", [TPC, V], BF16, isOutput=True)

    with tile.TileContext(nc) as tc:
        with ExitStack() as ctxA:
            consts = ctxA.enter_context(tc.tile_pool(name="consts", bufs=1))
            state = ctxA.enter_context(tc.tile_pool(name="state", bufs=3))
            zpool = ctxA.enter_context(tc.tile_pool(name="z", bufs=2))
            hscp = ctxA.enter_context(tc.tile_pool(name="hsc", bufs=2))
            hTp = ctxA.enter_context(tc.tile_pool(name="hT", bufs=2))
            wp = ctxA.enter_context(tc.tile_pool(name="w", bufs=8))
            hwp = ctxA.enter_context(tc.tile_pool(name="hw", bufs=2))
            outp = ctxA.enter_context(tc.tile_pool(name="outstg", bufs=4))
            gbp = None
            if use_gb:
                gbp = ctxA.enter_context(tc.tile_pool(name="gb", bufs=2))
            smp = ctxA.enter_context(tc.tile_pool(name="small", bufs=16))
            psY = ctxA.enter_context(
                tc.tile_pool(name="psY", bufs=2 * NCH, space="PSUM")
            )

            eps_t = consts.tile([128, 1], F32)
            nc.sync.dma_start(eps_t[:], eps_d[:])
            ident = consts.tile([128, 128], F32)
            nc.sync.dma_start(ident[:], ident_d[:])

            def load_w(l):
                """Layer weights in 4-ktile quarters — fine granularity lets
                each layer's first matmuls start before the whole 4.2 MB
                layer arrives (the early phase is DMA-bandwidth-starved).
                All on the scalar queue: mixing weight DMAs onto the sync
                queue alongside XBAR transposes corrupts transfers.
                """
                qs = []
                for qf in range(4):
                    wt = wp.tile([128, KT // 4, H], FP8, tag="w", name=f"w{l}_{qf}")
                    # layers 0-3: split quarters across BOTH queues — the
                    # early phase is bandwidth-starved and the sync queue
                    # carries no XBARs until ~t=115us (EARLY_PE covers the
                    # overlap window; concurrent XBAR+bulk on sync corrupts)
                    eng = nc.sync if (l <= 3 and qf % 2 == 1) else nc.scalar
                    eng.dma_start(
                        wt[:],
                        w_[l, qf * (KT // 4) : (qf + 1) * (KT // 4)].rearrange(
                            "k p o -> p k o"
                        ),
                    )
                    qs.append(wt)
                return qs

            h_cur = []
            hT = []
            for t in range(TT):
                # layer-0 transposed input comes precomputed from the host;
                # emitted BEFORE the weight quarters so the sync-queue ones
                # don't delay it (it gates the very first matmul)
                ht = hTp.tile([128, KT, 128], FP16, tag="hT", name=f"hTp{t}")
                nc.sync.dma_start(ht[:], h0T[t])
                hT.append(ht)
                st = state.tile([128, H], BF16, name=f"hinit{t}", tag="state")
                h_cur.append(st)

            w_half0 = load_w(0)
            # residual loads: plenty of slack (first needed ~35us in)
            for t in range(TT):
                nc.scalar.dma_start(h_cur[t][:], h0[t])
            w_half1 = load_w(1)

            def cast_transpose(src_f32, scale_imm, pool, name, t):
                """h [128tok, H] f32 -> hT [128, KT, 128tok] fp16 * scale."""
                hsc = hscp.tile([128, H], FP16, tag="hsc", name=f"hsc{name}")
                nc.vector.tensor_scalar_mul(hsc[:], src_f32[:], float(scale_imm))
                dst = pool.tile([128, KT, 128], FP16, tag="hT", name=f"hT{name}")
                # layer XBARs all ride sync (no weight traffic there, so no
                # ring-corruption hazard, and they stop delaying the weight
                # quarter issues queued on scalar). Tile 1's FINAL transpose
                # stays on scalar: it completes early in the head phase and
                # must not overlap the head-weight DMAs starting on sync.
                eng = nc.scalar if name == "fin1" else nc.sync
                eng.dma_start(dst[:], hsc[:], transpose=True)
                return dst

            def pe_transpose(src_f32, scale_imm, pool, name):
                """PE-based transpose via psY chunks — used for the early
                layers, when the DMA queues are bandwidth-starved."""
                dst = pool.tile([128, KT, 128], FP16, tag="hT", name=f"hT{name}")
                for j in range(NCH):
                    pc = psY.tile([128, CH], F32, tag="psY", name=f"pT{name}_{j}")
                    for u in range(CH // 128):
                        kt = j * (CH // 128) + u
                        nc.tensor.transpose(
                            pc[:, u * 128 : (u + 1) * 128],
                            src_f32[:, kt * 128 : (kt + 1) * 128],
                            ident[:],
                        )
                    nc.scalar.activation(
                        dst[:, j * (CH // 128) : (j + 1) * (CH // 128), :],
                        pc[:],
                        AF.Copy,
                        scale=float(scale_imm),
                    )
                return dst

            def ln_finish(affine_src, S_ap, SS_ap, g_t, b_t, name):
                S = smp.tile([128, 1], F32, tag="s0", name=f"S{name}")
                SS = smp.tile([128, 1], F32, tag="s1", name=f"SS{name}")
                nc.vector.tensor_reduce(S[:], S_ap, axis=AX.X, op=OP.add)
                nc.vector.tensor_reduce(SS[:], SS_ap, axis=AX.X, op=OP.add)
                negmean = smp.tile([128, 1], F32, tag="s2", name=f"nm{name}")
                nc.vector.tensor_scalar_mul(negmean[:], S[:], -1.0 / H)
                msq = smp.tile([128, 1], F32, tag="s3", name=f"msq{name}")
                nc.vector.tensor_scalar_mul(msq[:], SS[:], 1.0 / H)
                var = smp.tile([128, 1], F32, tag="s4", name=f"var{name}")
                nc.vector.tensor_tensor(var[:], negmean[:], negmean[:], OP.mult)
                nc.vector.tensor_tensor(var[:], msq[:], var[:], OP.subtract)
                std = smp.tile([128, 1], F32, tag="s5", name=f"std{name}")
                nc.scalar.activation(std[:], var[:], AF.Sqrt, bias=eps_t[:])
                rstd = smp.tile([128, 1], F32, tag="s6", name=f"rstd{name}")
                nc.vector.reciprocal(rstd[:], std[:])
                hn = state.tile([128, H], F32, tag="state", name=f"h{name}")
                nc.vector.tensor_scalar(
                    hn[:], affine_src[:], negmean[:], rstd[:], OP.add, OP.mult
                )
                if g_t is not None:
                    nc.vector.tensor_tensor(hn[:], hn[:], g_t[:], OP.mult)
                    nc.vector.tensor_tensor(hn[:], hn[:], b_t[:], OP.add)
                return hn

            fg = fb = None
            if use_gb:
                fg = gbp.tile([128, H], BF16, tag="g", name="gfin")
                nc.scalar.dma_start(fg[:], fing[None, :].to_broadcast((128, H)))
                fb = gbp.tile([128, H], BF16, tag="b", name="bfin")
                nc.scalar.dma_start(fb[:], finb[None, :].to_broadcast((128, H)))

            hTfin = [None] * TT

            def emit_final(t):
                """Final LN -> transposed head input for tile t.

                When the final affine is trivial (this input), LN of an
                LN output is the identity to ~1e-6 relative — skip it and
                transpose the layer-8 LN output directly.
                """
                h8 = h_cur[t]
                if not use_gb:
                    hTfin[t] = cast_transpose(h8, head_sigma, hTp, f"fin{t}", t)
                    return
                if t == 0:
                    fsums = smp.tile(
                        [128, 1 + NCH], F32, tag="fsums", name=f"smfin{t}"
                    )
                    nc.vector.tensor_reduce(
                        fsums[:, 0:1], h8[:], axis=AX.X, op=OP.add
                    )
                    for i in range(NCH):
                        dump = psY.tile(
                            [128, CH], F32, tag="psY", name=f"dmp{t}_{i}"
                        )
                        nc.scalar.activation(
                            dump[:],
                            h8[:, i * CH : (i + 1) * CH],
                            AF.Square,
                            accum_out=fsums[:, 1 + i : 2 + i],
                        )
                    hfin = ln_finish(
                        h8, fsums[:, 0:1], fsums[:, 1 : 1 + NCH],
                        fg, fb, f"fin{t}",
                    )
                else:
                    fsums = smp.tile([128, 2], F32, tag="fsums1", name=f"smfin{t}")
                    nc.vector.tensor_reduce(
                        fsums[:, 0:1], h8[:], axis=AX.X, op=OP.add
                    )
                    fsq = zpool.tile([128, H], F32, tag="z", name=f"fsq{t}")
                    nc.vector.tensor_tensor(fsq[:], h8[:], h8[:], OP.mult)
                    nc.vector.tensor_reduce(
                        fsums[:, 1:2], fsq[:], axis=AX.X, op=OP.add
                    )
                    hfin = ln_finish(
                        h8, fsums[:, 0:1], fsums[:, 1:2], fg, fb, f"fin{t}"
                    )
                hTfin[t] = cast_transpose(hfin, head_sigma, hTp, f"fin{t}", t)

            EARLY_PE = 4  # transposes for layers 1..4 run on the PE — the
            # DMA queues are bandwidth-starved while the weight prefetch
            # backlog drains
            pending = [None, None]
            gbt = {}
            hw_pre = []  # first head chunks, prefetched under the last layer
            for l in range(L):
                if l > 1:
                    w_half = load_w(l)
                elif l == 1:
                    w_half = w_half1
                else:
                    w_half = w_half0
                if l == L - 1:
                    for ch in range(2):
                        hwt = hwp.tile(
                            [128, KT, CV], FP8, tag="hw", name=f"hw{ch}"
                        )
                        nc.scalar.dma_start(
                            hwt[:],
                            hw_[:, :, ch * CV : (ch + 1) * CV].rearrange(
                                "k p v -> p k v"
                            ),
                        )
                        hw_pre.append(hwt)
                if use_gb:
                    g_t = gbp.tile([128, H], BF16, tag="g", name=f"g{l}")
                    nc.scalar.dma_start(g_t[:], lng[l][None, :].to_broadcast((128, H)))
                    b_t = gbp.tile([128, H], BF16, tag="b", name=f"b{l}")
                    nc.scalar.dma_start(b_t[:], lnb[l][None, :].to_broadcast((128, H)))
                    bias_t = gbp.tile([128, H], BF16, tag="bias", name=f"bias{l}")
                    nc.scalar.dma_start(
                        bias_t[:], lbias[l][None, :].to_broadcast((128, H))
                    )
                    gbt = dict(g=g_t, b=b_t, bias=bias_t)

                for t in range(TT):
                    hTt = hT[t]
                    ps = []
                    for i in range(NCH):
                        p = psY.tile([128, CH], F32, tag="psY", name=f"ps{l}_{t}_{i}")
                        ps.append(p)
                    for kt in range(KT):
                        if kt == KT // 2 and pending[1 - t] is not None:
                            pending[1 - t]()
                            pending[1 - t] = None
                        wt = w_half[kt // (KT // 4)]
                        for i in range(NCH):
                            nc.tensor.matmul(
                                ps[i][:],
                                lhsT=hTt[:, kt, :],
                                rhs=wt[:, kt % (KT // 4), i * CH : (i + 1) * CH],
                                start=(kt == 0),
                                stop=(kt == KT - 1),
                                skip_group_check=True,
                            )
                    z = zpool.tile([128, H], F32, tag="z", name=f"z{l}_{t}")
                    sums = smp.tile([128, 1 + NCH], F32, tag="sums", name=f"sm{l}_{t}")
                    resid = h_cur[t]
                    if use_gb:
                        hb = zpool.tile([128, H], F32, tag="hb", name=f"hb{l}_{t}")
                        nc.vector.tensor_tensor(hb[:], h_cur[t][:], gbt["bias"][:], OP.add)
                        resid = hb
                    for i in range(NCH):
                        nc.vector.tensor_add(
                            z[:, i * CH : (i + 1) * CH],
                            ps[i][:],
                            resid[:, i * CH : (i + 1) * CH],
                        )
                    nc.vector.tensor_reduce(sums[:, 0:1], z[:], axis=AX.X, op=OP.add)
                    for i in range(NCH):
                        nc.scalar.activation(
                            ps[i][:],
                            z[:, i * CH : (i + 1) * CH],
                            AF.Square,
                            accum_out=sums[:, 1 + i : 2 + i],
                        )
                    h_cur[t] = ln_finish(
                        z, sums[:, 0:1], sums[:, 1 : 1 + NCH],
                        gbt.get("g"), gbt.get("b"), f"{l}_{t}",
                    )
                    if l + 1 < L:
                        if l + 1 <= EARLY_PE:
                            def mk(tt, ll, src):
                                def emit():
                                    hT[tt] = pe_transpose(
                                        src, sigmas[ll + 1], hTp, f"{ll + 1}_{tt}"
                                    )
                                return emit
                            pending[t] = mk(t, l, h_cur[t])
                        else:
                            # emitted NOW: the cast sits right behind this
                            # tile's LN in the vector FIFO and the XBAR runs
                            # during the other tile's matmuls
                            hT[t] = cast_transpose(
                                h_cur[t], sigmas[l + 1], hTp, f"{l + 1}_{t}", t
                            )
                    else:
                        emit_final(t)

            # head: stream the full lm_head in CV-wide vocab chunks; each
            # chunk serves both token tiles (compute:DMA ~ 2.3:1)
            for ch in range(NCHK):
                if ch < 2:
                    hwt = hw_pre[ch]
                else:
                    # head weights stream on the otherwise-idle sync queue so
                    # their issue never queues behind the PSUM->staging
                    # copies (which block awaiting matmul completion)
                    hwt = hwp.tile([128, KT, CV], FP8, tag="hw", name=f"hw{ch}")
                    nc.sync.dma_start(
                        hwt[:],
                        hw_[:, :, ch * CV : (ch + 1) * CV].rearrange("k p v -> p k v"),
                    )
                for t in range(TT):
                    pss = [
                        psY.tile([128, CH], F32, tag="psY", name=f"ph{ch}_{t}_{v}")
                        for v in range(NVC)
                    ]
                    for kt in range(KT):
                        for vi in range(NVC):
                            nc.tensor.matmul(
                                pss[vi][:, 0:NV],
                                lhsT=hTfin[t][:, kt, :],
                                rhs=hwt[:, kt, vi * NV : (vi + 1) * NV],
                                start=(kt == 0),
                                stop=(kt == KT - 1),
                                skip_group_check=True,
                            )
                    o_t = outp.tile([128, CV], BF16, tag="ostg", name=f"o{ch}_{t}")
                    last = ch == NCHK - 1 and t == TT - 1
                    for vi in range(NVC):
                        dst = o_t[:, vi * NV : (vi + 1) * NV]
                        if last and vi % 2 == 1:
                            # final unit: split copies across both engines and
                            # drain in two DMAs so the kernel tail is shorter
                            nc.vector.tensor_scalar_mul(dst, pss[vi][:, 0:NV], 1.0)
                        else:
                            nc.scalar.copy(dst, pss[vi][:, 0:NV])
                        if last and vi == 1:
                            nc.scalar.dma_start(
                                out[t * 128 : (t + 1) * 128,
                                    ch * CV : ch * CV + 2 * NV],
                                o_t[:, 0 : 2 * NV],
                            )
                    if last:
                        nc.scalar.dma_start(
                            out[t * 128 : (t + 1) * 128,
                                ch * CV + 2 * NV : (ch + 1) * CV],
                            o_t[:, 2 * NV : CV],
                        )
                    else:
                        # out rides scalar, directly behind its own copies
                        nc.scalar.dma_start(
                            out[t * 128 : (t + 1) * 128, ch * CV : (ch + 1) * CV],
                            o_t[:],
                        )

    return nc


def _ternary(wmat):
    """Exact {-1,0,1} ternary tensor + fp32 scale, matching the reference."""
    w = np.asarray(wmat, dtype=np.float32)
    s = np.mean(np.abs(w), dtype=np.float32)
    t = np.clip(np.rint(w / (s + np.float32(1e-8))), -1.0, 1.0).astype(np.float32)
    return t, float(s)


def _split_scale(s):
    """s = sigma * 2^-e with sigma ~ 1 and 2^-e exact in fp8e4."""
    e = int(np.clip(np.round(-np.log2(s)), -7, 9))
    return s * (2.0**e), e


_NC_CACHE = {}
_LAST_RESULTS = None


def kernel(**inputs):
    global _LAST_RESULTS
    cfg = CFG_FULL
    L, H, NTOK, NC, TT, V = (
        cfg["L"], cfg["H"], cfg["NTOK"], cfg["NC"], cfg["TT"], cfg["V"],
    )
    KT = H // 128
    TPC = TT * 128  # tokens per core
    BF = ml_dtypes.bfloat16
    F8 = ml_dtypes.float8_e4m3fn

    ids = np.asarray(inputs["input_ids"]).astype(np.int64).reshape(-1)
    embed = np.asarray(inputs["embed"], dtype=np.float32)
    layer_w = np.asarray(inputs["layer_w"], dtype=np.float32)
    layer_b = np.asarray(inputs["layer_b"], dtype=np.float32)
    ln_g = np.asarray(inputs["ln_g"], dtype=np.float32)
    ln_b = np.asarray(inputs["ln_b"], dtype=np.float32)
    final_g = np.asarray(inputs["final_g"], dtype=np.float32)
    final_b = np.asarray(inputs["final_b"], dtype=np.float32)
    head_w = np.asarray(inputs["head_w"], dtype=np.float32)

    use_gb = not (
        np.all(layer_b == 0.0)
        and np.all(ln_g == 1.0)
        and np.all(ln_b == 0.0)
        and np.all(final_g == 1.0)
        and np.all(final_b == 0.0)
    )

    h0_full = embed[ids]  # [NTOK, H] fp32

    sigmas = []
    wT = np.empty([L, KT, 128, H], dtype=F8)
    for l in range(L):
        t, s = _ternary(layer_w[l])
        sig, e = _split_scale(s)
        sigmas.append(sig)
        wT[l] = (
            (np.ascontiguousarray(t.T) * np.float32(2.0**-e))
            .reshape(KT, 128, H)
            .astype(F8)
        )
    th, head_scale = _ternary(head_w)
    head_sigma, e_h = _split_scale(head_scale)
    headT = (
        (np.ascontiguousarray(th.T) * np.float32(2.0**-e_h))
        .reshape(KT, 128, -1)
        .astype(F8)
    )  # [KT, 128, V]

    key = (id(cfg), tuple(sigmas), head_sigma, use_gb)
    if key not in _NC_CACHE:
        _NC_CACHE.clear()
        nc = build_nc(cfg, sigmas, head_sigma, use_gb)
        # Bacc.finalize runs the TRN2 legalization passes (1-wait-per-
        # instruction event-semaphore split, matmul->ldweights wait motion,
        # register allocation). The PJRT exec path serializes nc as-is.
        nc.finalize()
        _NC_CACHE[key] = nc
    nc = _NC_CACHE[key]

    common = {
        "w": wT,
        "hw": headT,
        "ident": np.eye(128, dtype=np.float32),
        "eps": np.full((128, 1), EPS, np.float32),
    }
    if use_gb:
        common.update(
            lng=ln_g.astype(BF),
            lnb=ln_b.astype(BF),
            lbias=layer_b.astype(BF),
            fing=final_g.astype(BF),
            finb=final_b.astype(BF),
        )
    in_maps = []
    for c in range(NC):
        h0c = np.ascontiguousarray(
            h0_full[c * TPC : (c + 1) * TPC].reshape(TT, 128, H)
        )
        # layer-0 transposed+scaled input, in XBAR block layout
        h0t = np.ascontiguousarray(
            (h0c * np.float32(sigmas[0]))
            .transpose(0, 2, 1)  # [TT, H, 128tok]
            .reshape(TT, KT, 128, 128)
            .transpose(0, 2, 1, 3)  # [TT, 128p, KT, 128tok]
        ).astype(np.float16)
        in_maps.append(dict(common, h0=h0c.astype(BF), h0T=h0t))

    trace = bool(int(os.environ.get("TRIKERNEL_TRACE", "0")))
    res = run_bass_kernel_spmd(nc, in_maps, core_ids=list(range(NC)), trace=trace)
    _LAST_RESULTS = res

    full = np.concatenate(
        [np.asarray(res.results[c]["out"]) for c in range(NC)], axis=0
    )  # [NTOK, V] bf16
    return full.reshape(2, 1024, 32000).astype(np.float32)
